# revision 27
# baseline (speedup 1.0000x reference)
"""AttnRNN decoder kernel for trn2 (8 NeuronCores, data-parallel over batch).

Structure:
  host   : embedding gather, weight transposes, batch sharding (B=32 -> 4/core)
  device : phase0  seq_qT / m2T / keyT_enc precompute (t-independent terms)
           phase1  127 sequential steps, fully column-form (features on
                   partitions, batch=4 moving dim; no transposes; sigmoid via
                   exp+reciprocal so only one act-table set is used)
           phase2  hoisted vocab projection (bf16) + log_softmax + DMA out
"""

import numpy as np

import concourse.bass as bass
import concourse.bacc as bacc
import concourse.mybir as mybir
import concourse.tile as tile
from concourse.bass_utils import run_bass_kernel_spmd

F32 = mybir.dt.float32
BF16 = mybir.dt.bfloat16
AF = mybir.ActivationFunctionType

B, L, D, T = 32, 512, 256, 128
H, NCOM, NB = 256, 8000, 128
V = NCOM + NB
NCORES = 8
BS = B // NCORES          # 4 examples per core
TT = T - 1                # 127 decode steps

# vocab chunking for phase 2: 15x512 + 1x320 common, then 128 batched
VCH = [(i * 512, 512) for i in range(15)] + [(7680, 320)]


def _pmajor(x, nchunk):
    """(nchunk*128, ...) -> (128, nchunk, ...) partition-major."""
    s = x.shape
    return np.ascontiguousarray(
        x.reshape(nchunk, 128, *s[1:]).transpose(1, 0, *range(2, 1 + len(s)))
    )


def _build(cfg):
    """Build the single-core program (SPMD-replicated across 8 cores)."""
    nc = bacc.Bacc("TRN2", target_bir_lowering=False, debug=False)

    dr = {}

    def din(name, shape, dt=F32):
        dr[name] = nc.dram_tensor(name, list(shape), dt, kind="ExternalInput").ap()
        return dr[name]

    enc_r = din("enc_r", (128, BS, 4, D), BF16)    # enc[b, 128*lc+p, d]
    enct = din("enct", (128, 2, BS, L), BF16)      # enc[b, l, 128*dc+p]
    tgtt = din("tgtt", (128, 2, TT, BS), BF16)     # tgt[b, t, 128*dc+p]
    cembt = din("cembt", (128, 2, NCOM), BF16)     # common[v, 128*dc+p]
    obt = din("obt", (128, 2, BS, NB), BF16)       # batched[b, v, 128*dc+p]
    wqt = din("wqt", (128, 2, D), BF16)                  # Wq[e, 128*dc+p]
    wket = din("wket", (128, 2, D), BF16)
    wkh2 = din("wkh2", (128, 2, D), BF16)   # akw[e, 256+f] with e on partitions
    cwt = din("cwt", (128, 4, H), BF16)                  # combine_w[g, 128*fc+p]
    lwt = din("lwt", (128, 4, 4 * H), BF16)              # [W_ih.T ; W_hh.T], ifog->ifog perm
    owt = din("owt", (128, 2, D), BF16)
    onesq = din("onesq", (128, 128), BF16)
    if cfg["any_bias"]:
        bqc = din("bqc", (128, 2))
        bkc = din("bkc", (128, 2))
        brow = din("brow", (1, 2 * H + 4 * H + D), BF16)   # [bc(256), bl(1024, ifog-perm), bo(256)]
        ones = din("ones", (1, BS), BF16)
    if cfg["enc_mask"]:
        emadd = din("emadd", (128, BS, 4))
    if cfg["out_mask"]:
        bmr = din("bmr", (BS, NB), BF16)
        onest = din("onest", (1, TT), BF16)

    out = nc.dram_tensor("out", [BS, TT, V], F32, kind="ExternalOutput").ap()
    if cfg.get("dbg_lin"):
        dbglin = nc.dram_tensor("dbglin", [128, 2, BS, TT], BF16, kind="ExternalOutput").ap()
        dr_dbg = {}
        for nm_, sh_, dt_ in [("dbg_wt", [128, BS, 4, TT], BF16),
                              ("dbg_c", [128, 2, BS, TT], F32),
                              ("dbg_h", [128, 2, BS, TT], BF16),
                              ("dbg_scp", [128, BS, 4, TT], F32),
                              ("dbg_ket", [128, 2, TT, BS], BF16),
                              ("dbg_seqq", [128, 2, BS, L], BF16),
                              ("dbg_m2", [128, 2, BS, L], BF16)]:
            dr_dbg[nm_] = nc.dram_tensor(nm_, sh_, dt_, kind="ExternalOutput").ap()

    with tile.TileContext(nc) as tc:
        with (
            tc.tile_pool(name="const", bufs=1) as kc,
            tc.tile_pool(name="state", bufs=3) as stp,
        ):
            # ---- persistent SBUF loads (ordered to unblock phase 0/1) ----
            enct_sb0 = kc.tile([128, 2, BS, L], BF16)
            nc.sync.dma_start(enct_sb0[:], enct[:])
            wqt_sb = kc.tile([128, 2, D], BF16)
            nc.sync.dma_start(wqt_sb[:], wqt[:])
            wket_sb = kc.tile([128, 2, D], BF16)
            nc.sync.dma_start(wket_sb[:], wket[:])
            tgtt_sb = kc.tile([128, 2, TT, BS], BF16)
            nc.sync.dma_start(tgtt_sb[:], tgtt[:])
            wkh2_sb = kc.tile([128, 2, D], BF16)
            nc.sync.dma_start(wkh2_sb[:], wkh2[:])
            cwt_sb = kc.tile([128, 4, H], BF16)
            nc.sync.dma_start(cwt_sb[:], cwt[:])
            lwt_sb = kc.tile([128, 4, 4 * H], BF16)
            nc.sync.dma_start(lwt_sb[:], lwt[:])
            owt_sb = kc.tile([128, 2, D], BF16)
            nc.sync.dma_start(owt_sb[:], owt[:])
            onesq_sb = kc.tile([128, 128], BF16)
            nc.sync.dma_start(onesq_sb[:], onesq[:])
            enc_sb = kc.tile([128, BS, 4, D], BF16)
            nc.sync.dma_start(enc_sb[:], enc_r[:])
            # dummy exp: pulls the act-table load off the step-0 critical path
            warm = kc.tile([1, 1], F32)
            nc.scalar.activation(warm[:], onesq_sb[0:1, 0:1], AF.Exp)
            if cfg["any_bias"]:
                bqc_sb = kc.tile([128, 2], F32)
                nc.sync.dma_start(bqc_sb[:], bqc[:])
                bkc_sb = kc.tile([128, 2], F32)
                nc.sync.dma_start(bkc_sb[:], bkc[:])
                brow_sb = kc.tile([1, 2 * H + 4 * H + D], BF16)
                nc.sync.dma_start(brow_sb[:], brow[:])
                ones_sb = kc.tile([1, BS], BF16)
                nc.sync.dma_start(ones_sb[:], ones[:])
            if cfg["enc_mask"]:
                emadd_sb = kc.tile([128, BS, 4], F32)
                nc.sync.dma_start(emadd_sb[:], emadd[:])
            if cfg["out_mask"]:
                bmr_sb = kc.tile([BS, NB], BF16)
                nc.sync.dma_start(bmr_sb[:], bmr[:])
                onest_sb = kc.tile([1, TT], BF16)
                nc.sync.dma_start(onest_sb[:], onest[:])

            seqqt_sb = kc.tile([128, 2, BS, L], BF16)
            m2t_sb = kc.tile([128, 2, BS, L], BF16)
            ket_sb = kc.tile([128, 2, TT, BS], BF16)
            linT_sb = kc.tile([128, 2, BS, TT], BF16)

            # ---- phase 0: seq_qT, m2T, keyT_enc ----
            with (
                tc.tile_pool(name="enctp", bufs=1) as ep,
                tc.tile_pool(name="p0ps", bufs=2, space="PSUM") as p0,
            ):
                enct_sb = enct_sb0
                for b in range(BS):
                    for c in range(2):
                        ps = p0.tile([128, 512], F32)
                        for k in range(2):
                            nc.tensor.matmul(
                                ps[:],
                                wqt_sb[:, k, c * 128:(c + 1) * 128],
                                enct_sb[:, k, b, :],
                                start=(k == 0), stop=(k == 1),
                            )
                        if cfg["any_bias"]:
                            nc.scalar.activation(
                                seqqt_sb[:, c, b, :], ps[:], AF.Identity,
                                bias=bqc_sb[:, c:c + 1],
                            )
                        else:
                            nc.vector.tensor_copy(seqqt_sb[:, c, b, :], ps[:])
                for c in range(2):
                    ps = p0.tile([128, 512], F32)
                    for k in range(2):
                        nc.tensor.matmul(
                            ps[:, 0:TT * BS],
                            wket_sb[:, k, c * 128:(c + 1) * 128],
                            tgtt_sb[:, k, :, :],
                            start=(k == 0), stop=(k == 1),
                        )
                    if cfg["any_bias"]:
                        nc.scalar.activation(
                            ket_sb[:, c, :, :], ps[:, 0:TT * BS], AF.Identity,
                            bias=bkc_sb[:, c:c + 1],
                        )
                    else:
                        nc.vector.tensor_copy(ket_sb[:, c, :, :], ps[:, 0:TT * BS])

            # phase-2-only tensors: DMA'd after the phase-0 loads so they
            # don't delay phase 0/1 startup
            cembt_sb = kc.tile([128, 2, NCOM], BF16)
            nc.sync.dma_start(cembt_sb[:], cembt[:])
            obt_sb = kc.tile([128, 2, BS, NB], BF16)
            nc.sync.dma_start(obt_sb[:], obt[:])

            # ---- phase 1: 127 sequential steps, column form ----
            nb_ = 1 if cfg["any_bias"] else 0
            with (
                tc.tile_pool(name="scps", bufs=2, space="PSUM") as scps,
                tc.tile_pool(name="gpsp", bufs=1, space="PSUM") as gpsp,
                tc.tile_pool(name="smps", bufs=1, space="PSUM") as smps,
                tc.tile_pool(name="misc", bufs=1, space="PSUM") as mps,
                tc.tile_pool(name="m2ps", bufs=1, space="PSUM") as pm,
                tc.tile_pool(name="work", bufs=3) as sbw,
            ):
                hT_cur = None

                def lin_mms(hT):
                    lps = mps.tile([128, 2, BS], F32, tag="lps")
                    for c in range(2):
                        for k in range(2):
                            nc.tensor.matmul(
                                lps[:, c, :],
                                owt_sb[:, k, c * 128:(c + 1) * 128],
                                hT[:, k, :],
                                start=(k == 0), stop=(k == 1 and nb_ == 0),
                            )
                        if nb_:
                            nc.tensor.matmul(
                                lps[:, c, :],
                                brow_sb[0:1, 5 * H + c * 128:5 * H + (c + 1) * 128],
                                ones_sb[0:1, :],
                                start=False, stop=True,
                            )
                    return lps

                def m2_mms(b):
                    # m2T[:, :, b, :] is first needed by step b+1's h-half;
                    # computing example b's slice at the end of step b keeps
                    # step 0 from waiting on it and spreads the copies out
                    for c in range(2):
                        ps = pm.tile([128, 512], F32, tag="m2p")
                        for k in range(2):
                            nc.tensor.matmul(
                                ps[:],
                                wkh2_sb[:, k, c * 128:(c + 1) * 128],
                                seqqt_sb[:, k, b, :],
                                start=(k == 0), stop=(k == 1),
                            )
                        eng = nc.vector if c == 0 else nc.scalar
                        if c == 0:
                            nc.vector.tensor_copy(m2t_sb[:, c, b, :], ps[:])
                        else:
                            nc.scalar.activation(
                                m2t_sb[:, c, b, :], ps[:], AF.Identity
                            )

                for t in range(0 if cfg.get("skip_p1") else TT):
                    # -- scores (psum col [b, lc]): enc half has no h dep --
                    # NOTE: one accumulation group for the whole tile per step
                    # (start only on the first mm, stop only on the last):
                    # interleaved per-column groups in one psum zero region
                    # corrupt each other (start re-marks the region pending-
                    # zero, so open columns lose their partial sums).
                    scp = scps.tile([128, BS, 4], F32, tag="scp")
                    for b in range(BS):
                        for lc in range(4):
                            col = scp[:, b, lc:lc + 1]
                            for k in range(2):
                                nc.tensor.matmul(
                                    col,
                                    seqqt_sb[:, k, b, lc * 128:(lc + 1) * 128],
                                    ket_sb[:, k, t, b:b + 1],
                                    start=(b == 0 and lc == 0 and k == 0),
                                    stop=(t == 0 and b == BS - 1 and lc == 3 and k == 1),
                                )
                    if t > 0:
                        for b in range(BS):
                            for lc in range(4):
                                col = scp[:, b, lc:lc + 1]
                                for k in range(2):
                                    nc.tensor.matmul(
                                        col,
                                        m2t_sb[:, k, b, lc * 128:(lc + 1) * 128],
                                        hT_cur[:, k, b:b + 1],
                                        start=False,
                                        stop=(b == BS - 1 and lc == 3 and k == 1),
                                    )
                    # gates h-half early (off critical path); single
                    # accumulation group per step (see scores note)
                    gp = gpsp.tile([128, 8, BS], F32, tag="gp")
                    if t > 0:
                        for g in range(8):
                            for k in range(2):
                                nc.tensor.matmul(
                                    gp[:, g, :],
                                    lwt_sb[:, 2 + k, g * 128:(g + 1) * 128],
                                    hT_cur[:, k, :],
                                    start=(g == 0 and k == 0), stop=False,
                                )
                        # output projection for step t-1 (h_{t-1} ready now)
                        lps = lin_mms(hT_cur)
                        nc.vector.tensor_copy(linT_sb[:, :, :, t - 1], lps[:])

                    if cfg["enc_mask"]:
                        nc.vector.tensor_add(scp[:], scp[:], emadd_sb[:])

                    # -- softmax: exp -> sums (4 accumulated ones-matmuls,
                    #    replicated over partitions) -> recip -> scale --
                    wt = sbw.tile([128, BS, 4], BF16, tag="wt")
                    nc.scalar.activation(wt[:], scp[:], AF.Exp)
                    smp = smps.tile([128, 1, BS], F32, tag="smp")
                    for lc in range(4):
                        nc.tensor.matmul(
                            smp[:, 0, :], onesq_sb[:], wt[:, :, lc],
                            start=(lc == 0), stop=(lc == 3),
                        )
                    # attention context, column form
                    atp = mps.tile([128, 2, BS], F32, tag="atp")
                    for b in range(BS):
                        for dc in range(2):
                            for lc in range(4):
                                nc.tensor.matmul(
                                    atp[:, dc, b:b + 1],
                                    enc_sb[:, b, lc, dc * 128:(dc + 1) * 128],
                                    wt[:, b, lc:lc + 1],
                                    start=(lc == 0), stop=(lc == 3),
                                )
                    rb = sbw.tile([128, 1, BS], F32, tag="rb")
                    nc.vector.reciprocal(rb[:, 0, :], smp[:, 0, :])
                    ats = sbw.tile([128, 2, BS], BF16, tag="ats")
                    nc.vector.tensor_mul(
                        ats[:], atp[:], rb[:].broadcast_to([128, 2, BS])
                    )

                    # -- combine + relu --
                    cbp = mps.tile([128, 2, BS], F32, tag="cbp")
                    cl = [tgtt_sb[:, 0, t, :], tgtt_sb[:, 1, t, :],
                          ats[:, 0, :], ats[:, 1, :]]
                    for fc in range(2):
                        for k in range(4):
                            nc.tensor.matmul(
                                cbp[:, fc, :],
                                cwt_sb[:, k, fc * 128:(fc + 1) * 128],
                                cl[k],
                                start=(k == 0), stop=(k == 3 and nb_ == 0),
                            )
                        if nb_:
                            nc.tensor.matmul(
                                cbp[:, fc, :],
                                brow_sb[0:1, fc * 128:(fc + 1) * 128],
                                ones_sb[0:1, :],
                                start=False, stop=True,
                            )
                    cbT = sbw.tile([128, 2, BS], BF16, tag="cbT")
                    nc.vector.tensor_scalar_max(cbT[:], cbp[:], 0.0)

                    # -- LSTM gates, comb half (accumulates into gp) --
                    for g in range(8):
                        for k in range(2):
                            nc.tensor.matmul(
                                gp[:, g, :],
                                lwt_sb[:, k, g * 128:(g + 1) * 128],
                                cbT[:, k, :],
                                start=(t == 0 and g == 0 and k == 0),
                                stop=(g == 7 and k == 1 and nb_ == 0),
                            )
                        if nb_:
                            nc.tensor.matmul(
                                gp[:, g, :],
                                brow_sb[0:1, 2 * H + g * 128:2 * H + (g + 1) * 128],
                                ones_sb[0:1, :],
                                start=False, stop=(g == 7),
                            )

                    # -- gate nonlinearities, one exp for all gates:
                    #    sigmoid(x) = 1/(1+exp(-x)); tanh(g) = 2*sigmoid(2g)-1
                    #    (g-gate weights are pre-doubled on the host) --
                    sie = sbw.tile([128, 8, BS], F32, tag="sie")
                    nc.scalar.activation(sie[:], gp[:], AF.Exp, scale=-1.0)
                    si = sbw.tile([128, 8, BS], F32, tag="si")
                    nc.vector.tensor_scalar_add(si[:], sie[:], 1.0)
                    nc.vector.reciprocal(si[:], si[:])
                    # gc holds [tanh(g) | c_{t-1}] so si[i,f] multiplies both
                    # in a single op: m12 = [si_i*tg | si_f*c]
                    gc = gc_next if t > 0 else stp.tile([128, 4, BS], F32, tag="gc")
                    nc.vector.tensor_scalar(
                        gc[:, 0:2, :], si[:, 6:8, :], 2.0, -1.0,
                        mybir.AluOpType.mult, mybir.AluOpType.add,
                    )

                    # -- c/h update --
                    gc_next = stp.tile([128, 4, BS], F32, tag="gc")
                    if t > 0:
                        m12 = sbw.tile([128, 4, BS], F32, tag="m12")
                        nc.vector.tensor_mul(m12[:], si[:, 0:4, :], gc[:])
                        c_new = gc_next[:, 2:4, :]
                        nc.vector.tensor_add(c_new, m12[:, 0:2, :], m12[:, 2:4, :])
                    else:
                        c_new = gc_next[:, 2:4, :]
                        nc.vector.tensor_mul(c_new, si[:, 0:2, :], gc[:, 0:2, :])
                    tc_ = sbw.tile([128, 2, BS], F32, tag="tc")
                    nc.scalar.activation(tc_[:], c_new, AF.Tanh)
                    hT_new = stp.tile([128, 2, BS], BF16, tag="hstate")
                    nc.vector.tensor_mul(hT_new[:], si[:, 4:6, :], tc_[:])

                    if cfg.get("dbg_lin"):
                        nc.sync.dma_start(dr_dbg["dbg_h"][:, :, :, t], hT_new[:])
                        nc.sync.dma_start(dr_dbg["dbg_c"][:, :, :, t], c_new[:])
                        nc.sync.dma_start(dr_dbg["dbg_wt"][:, :, :, t], wt[:])
                        scpc = sbw.tile([128, BS, 4], F32, tag="scpdbg")
                        nc.vector.tensor_copy(scpc[:], scp[:])
                        nc.sync.dma_start(dr_dbg["dbg_scp"][:, :, :, t], scpc[:])

                    hT_cur = hT_new
                    if t == 0:
                        for b_ in range(BS):
                            m2_mms(b_)

                if not cfg.get("skip_p1"):
                    lps = lin_mms(hT_cur)
                    nc.vector.tensor_copy(linT_sb[:, :, :, TT - 1], lps[:])

            if cfg.get("dbg_lin"):
                nc.sync.dma_start(dbglin[:], linT_sb[:])
                nc.sync.dma_start(dr_dbg["dbg_ket"][:], ket_sb[:])
                nc.sync.dma_start(dr_dbg["dbg_seqq"][:], seqqt_sb[:])
                nc.sync.dma_start(dr_dbg["dbg_m2"][:], m2t_sb[:])
            # ---- phase 2: vocab projection + log_softmax ----
            with (
                tc.tile_pool(name="p2ps", bufs=3, space="PSUM") as p2,
                tc.tile_pool(name="p2ps_b", bufs=3, space="PSUM") as p2b,
                tc.tile_pool(name="ep2", bufs=1) as ep2,
                tc.tile_pool(name="outst", bufs=3) as osp,
                tc.tile_pool(name="sm2", bufs=2) as sm2,
            ):
                nch = len(VCH) + 1

                def chunk_mms(b, j, ps):
                    if j < len(VCH):
                        off, w = VCH[j]
                        rhs = [cembt_sb[:, k, off:off + w] for k in range(2)]
                    else:
                        w = NB
                        rhs = [obt_sb[:, k, b, :] for k in range(2)]
                    masked = j == nch - 1 and cfg["out_mask"]
                    for k in range(2):
                        nc.tensor.matmul(
                            ps[:, 0:w],
                            linT_sb[:, k, b, :],
                            rhs[k],
                            start=(k == 0),
                            stop=(k == 1 and not masked),
                        )
                    if masked:
                        nc.tensor.matmul(
                            ps[:, 0:w], onest_sb[0:1, :],
                            bmr_sb[b:b + 1, :],
                            start=False, stop=True,
                        )
                    return w

                # pass 1: exp-sums only (Act); pass 2 recomputes the cheap
                # matmul and writes out = logits - log(sumexp) on DVE, so the
                # expensive Ln pass over the full vocab disappears.
                for b in range(0 if cfg.get("skip_p2") else BS):
                    ss = sm2.tile([TT, nch], F32, tag="ss")
                    for j in range(nch):
                        ps = p2.tile([TT, 512], F32, tag="p2")
                        w = chunk_mms(b, j, ps)
                        exs = ep2.tile([TT, 512], BF16, tag="exs", bufs=4)
                        nc.scalar.activation(
                            exs[:, 0:w], ps[:, 0:w], AF.Exp,
                            accum_out=ss[:, j:j + 1],
                        )
                    st = sm2.tile([TT, 1], F32, tag="st")
                    nc.vector.reduce_sum(
                        st[:], ss[:], axis=mybir.AxisListType.X
                    )
                    lz = sm2.tile([TT, 1], F32, tag="lz")
                    nc.scalar.activation(lz[:], st[:], AF.Ln)
                    nlz = sm2.tile([TT, 1], F32, tag="nlz")
                    nc.vector.tensor_scalar_mul(nlz[:], lz[:], -1.0)
                    for j in range(nch):
                        if j < len(VCH):
                            voff = VCH[j][0]
                        else:
                            voff = NCOM
                        ps = p2b.tile([TT, 512], F32, tag="p2b")
                        w = chunk_mms(b, j, ps)
                        ot = osp.tile([TT, 512], F32, tag="ot", bufs=8)
                        if b == BS - 1 and j % 2 == 1:
                            # last example has no next-example exp stream to
                            # overlap with: split subs between DVE and Act
                            nc.scalar.activation(
                                ot[:, 0:w], ps[:, 0:w], AF.Identity, bias=nlz[:],
                            )
                        else:
                            nc.vector.tensor_scalar_sub(ot[:, 0:w], ps[:, 0:w], lz[:])
                        nc.sync.dma_start(
                            out[b, :, voff:voff + w], ot[:, 0:w]
                        )

    nc.compile()
    return nc


_CACHE = {}


def kernel(**inputs):
    inp = {k: np.asarray(v) for k, v in inputs.items()}
    enc = inp["encoder_outputs"].astype(np.float32)
    encm = inp["encoder_outputs_mask"]
    ob = inp["output_batched_encodings"].astype(np.float32)
    obm = inp["output_batched_encodings_mask"]
    idx = inp["target_idxs"]
    cem = inp["common_embedding"].astype(np.float32)
    akw = inp["attn_key_w"].astype(np.float32)
    akb = inp["attn_key_b"].astype(np.float32)
    aqw = inp["attn_query_w"].astype(np.float32)
    aqb = inp["attn_query_b"].astype(np.float32)
    cw = inp["combine_w"].astype(np.float32)
    cb = inp["combine_b"].astype(np.float32)
    wih = inp["lstm_w_ih"].astype(np.float32)
    whh = inp["lstm_w_hh"].astype(np.float32)
    bih = inp["lstm_b_ih"].astype(np.float32)
    bhh = inp["lstm_b_hh"].astype(np.float32)
    ow = inp["out_w"].astype(np.float32)
    obias = inp["out_b"].astype(np.float32)

    # teacher-forced embedding gather (host: data-dependent indexing)
    is_c = idx < NCOM
    cidx = np.clip(idx, 0, NCOM - 1)
    bidx = np.clip(idx - NCOM, 0, NB - 1)
    ge_c = cem[cidx]                                   # (B, T, D)
    ge_b = np.take_along_axis(ob, bidx[..., None], axis=1)
    tgt = np.where(is_c[..., None], ge_c, ge_b)[:, :TT, :].astype(np.float32)

    any_bias = bool(
        np.any(akb) or np.any(aqb) or np.any(cb) or np.any(bih)
        or np.any(bhh) or np.any(obias)
    )
    enc_mask = not bool(encm.all())
    out_mask = not bool(obm.all())

    cfg = {"any_bias": any_bias, "enc_mask": enc_mask, "out_mask": out_mask}
    key = (any_bias, enc_mask, out_mask)
    if key not in _CACHE:
        _CACHE[key] = _build(cfg)
    nc = _CACHE[key]

    # lstm weights, gate order [i, f, o, g]; g doubled so one exp computes all
    # gates: tanh(g) = 2*sigmoid(2g)-1
    lcat = np.concatenate([wih.T, whh.T], axis=0)       # (2*H(g), 4*H)
    perm = np.concatenate(
        [lcat[:, 0:H], lcat[:, H:2 * H], lcat[:, 3 * H:4 * H],
         2.0 * lcat[:, 2 * H:3 * H]],
        axis=1,
    )

    # shared (replicated) tensors
    import ml_dtypes
    bft = ml_dtypes.bfloat16
    shared = {
        "cembt": _pmajor(np.ascontiguousarray(cem.T), 2).astype(bft),
        "wqt": _pmajor(np.ascontiguousarray(aqw.T), 2).astype(bft),
        "wket": _pmajor(np.ascontiguousarray(akw[:, :D].T), 2).astype(bft),
        "wkh2": _pmajor(np.ascontiguousarray(akw[:, D:]), 2).astype(bft),
        "cwt": _pmajor(np.ascontiguousarray(cw.T), 4).astype(bft),
        "lwt": _pmajor(np.ascontiguousarray(perm), 4).astype(bft),
        "owt": _pmajor(np.ascontiguousarray(ow.T), 2).astype(bft),
        "onesq": np.ones((128, 128), bft),
    }
    if any_bias:
        bl = bih + bhh
        blp = np.concatenate(
            [bl[0:H], bl[H:2 * H], bl[3 * H:4 * H], 2.0 * bl[2 * H:3 * H]]
        )
        shared["bqc"] = _pmajor(aqb, 2)
        shared["bkc"] = _pmajor(akb, 2)
        shared["brow"] = np.concatenate(
            [cb, blp, obias]
        )[None, :].astype(bft)
        shared["ones"] = np.ones((1, BS), bft)
    if out_mask:
        shared["onest"] = np.ones((1, TT), bft)

    in_maps = []
    for c in range(NCORES):
        sl = slice(c * BS, (c + 1) * BS)
        e = enc[sl]                                    # (BS, L, D)
        tg_ = tgt[sl]                                  # (BS, TT, D)
        obs = ob[sl]                                   # (BS, NB, D)
        m = dict(shared)
        m["enc_r"] = np.ascontiguousarray(
            e.reshape(BS, 4, 128, D).transpose(2, 0, 1, 3)
        ).astype(bft)
        m["enct"] = np.ascontiguousarray(
            e.transpose(2, 0, 1).reshape(2, 128, BS, L).transpose(1, 0, 2, 3)
        ).astype(bft)
        m["tgtt"] = np.ascontiguousarray(
            tg_.transpose(2, 1, 0).reshape(2, 128, TT, BS).transpose(1, 0, 2, 3)
        ).astype(bft)
        m["obt"] = np.ascontiguousarray(
            obs.transpose(2, 0, 1).reshape(2, 128, BS, NB).transpose(1, 0, 2, 3)
        ).astype(ml_dtypes.bfloat16)
        if enc_mask:
            em = np.where(encm[sl], 0.0, -1e30).astype(np.float32)  # (BS, L)
            m["emadd"] = np.ascontiguousarray(
                em.reshape(BS, 4, 128).transpose(2, 0, 1)
            )
        if out_mask:
            m["bmr"] = np.where(obm[sl], 0.0, -1e30).astype(bft)
        in_maps.append(m)

    res = run_bass_kernel_spmd(nc, in_maps, list(range(NCORES)))
    outs = [res.results[c]["out"].reshape(BS, TT, V) for c in range(NCORES)]
    return np.concatenate(outs, axis=0).astype(np.float32)


# revision 29
# speedup vs baseline: 1.0032x; 1.0032x over previous
"""AttnRNN decoder kernel for trn2 (8 NeuronCores, data-parallel over batch).

Structure:
  host   : embedding gather, weight transposes, batch sharding (B=32 -> 4/core)
  device : phase0  seq_qT / m2T / keyT_enc precompute (t-independent terms)
           phase1  127 sequential steps, fully column-form (features on
                   partitions, batch=4 moving dim; no transposes; sigmoid via
                   exp+reciprocal so only one act-table set is used)
           phase2  hoisted vocab projection (bf16) + log_softmax + DMA out
"""

import numpy as np

import concourse.bass as bass
import concourse.bacc as bacc
import concourse.mybir as mybir
import concourse.tile as tile
from concourse.bass_utils import run_bass_kernel_spmd

F32 = mybir.dt.float32
BF16 = mybir.dt.bfloat16
AF = mybir.ActivationFunctionType

B, L, D, T = 32, 512, 256, 128
H, NCOM, NB = 256, 8000, 128
V = NCOM + NB
NCORES = 8
BS = B // NCORES          # 4 examples per core
TT = T - 1                # 127 decode steps

# vocab chunking for phase 2: 15x512 + 1x320 common, then 128 batched
VCH = [(i * 512, 512) for i in range(15)] + [(7680, 320)]


def _pmajor(x, nchunk):
    """(nchunk*128, ...) -> (128, nchunk, ...) partition-major."""
    s = x.shape
    return np.ascontiguousarray(
        x.reshape(nchunk, 128, *s[1:]).transpose(1, 0, *range(2, 1 + len(s)))
    )


def _build(cfg):
    """Build the single-core program (SPMD-replicated across 8 cores)."""
    nc = bacc.Bacc("TRN2", target_bir_lowering=False, debug=False)

    dr = {}

    def din(name, shape, dt=F32):
        dr[name] = nc.dram_tensor(name, list(shape), dt, kind="ExternalInput").ap()
        return dr[name]

    enc_r = din("enc_r", (128, BS, 4, D), BF16)    # enc[b, 128*lc+p, d]
    enct = din("enct", (128, 2, BS, L), BF16)      # enc[b, l, 128*dc+p]
    tgtt = din("tgtt", (128, 2, TT, BS), BF16)     # tgt[b, t, 128*dc+p]
    cembt = din("cembt", (128, 2, NCOM), BF16)     # common[v, 128*dc+p]
    obt = din("obt", (128, 2, BS, NB), BF16)       # batched[b, v, 128*dc+p]
    wqt = din("wqt", (128, 2, D), BF16)                  # Wq[e, 128*dc+p]
    wket = din("wket", (128, 2, D), BF16)
    wkh2 = din("wkh2", (128, 2, D), BF16)   # akw[e, 256+f] with e on partitions
    cwt = din("cwt", (128, 4, H), BF16)                  # combine_w[g, 128*fc+p]
    lwt = din("lwt", (128, 4, 4 * H), BF16)              # [W_ih.T ; W_hh.T], ifog->ifog perm
    owt = din("owt", (128, 2, D), BF16)
    onesq = din("onesq", (128, 128), BF16)
    if cfg["any_bias"]:
        bqc = din("bqc", (128, 2))
        bkc = din("bkc", (128, 2))
        brow = din("brow", (1, 2 * H + 4 * H + D), BF16)   # [bc(256), bl(1024, ifog-perm), bo(256)]
        ones = din("ones", (1, BS), BF16)
    if cfg["enc_mask"]:
        emadd = din("emadd", (128, BS, 4))
    if cfg["out_mask"]:
        bmr = din("bmr", (BS, NB), BF16)
        onest = din("onest", (1, TT), BF16)

    out = nc.dram_tensor("out", [BS, TT, V], F32, kind="ExternalOutput").ap()
    if cfg.get("dbg_lin"):
        dbglin = nc.dram_tensor("dbglin", [128, 2, BS, TT], BF16, kind="ExternalOutput").ap()
        dr_dbg = {}
        for nm_, sh_, dt_ in [("dbg_wt", [128, BS, 4, TT], BF16),
                              ("dbg_c", [128, 2, BS, TT], F32),
                              ("dbg_h", [128, 2, BS, TT], BF16),
                              ("dbg_scp", [128, BS, 4, TT], F32),
                              ("dbg_ket", [128, 2, TT, BS], BF16),
                              ("dbg_seqq", [128, 2, BS, L], BF16),
                              ("dbg_m2", [128, 2, BS, L], BF16)]:
            dr_dbg[nm_] = nc.dram_tensor(nm_, sh_, dt_, kind="ExternalOutput").ap()

    with tile.TileContext(nc) as tc:
        with (
            tc.tile_pool(name="const", bufs=1) as kc,
            tc.tile_pool(name="state", bufs=3) as stp,
        ):
            # ---- persistent SBUF loads (ordered to unblock phase 0/1) ----
            enct_sb0 = kc.tile([128, 2, BS, L], BF16)
            nc.sync.dma_start(enct_sb0[:], enct[:])
            wqt_sb = kc.tile([128, 2, D], BF16)
            nc.sync.dma_start(wqt_sb[:], wqt[:])
            wket_sb = kc.tile([128, 2, D], BF16)
            nc.sync.dma_start(wket_sb[:], wket[:])
            tgtt_sb = kc.tile([128, 2, TT, BS], BF16)
            nc.sync.dma_start(tgtt_sb[:], tgtt[:])
            wkh2_sb = kc.tile([128, 2, D], BF16)
            nc.sync.dma_start(wkh2_sb[:], wkh2[:])
            cwt_sb = kc.tile([128, 4, H], BF16)
            nc.sync.dma_start(cwt_sb[:], cwt[:])
            lwt_sb = kc.tile([128, 4, 4 * H], BF16)
            nc.sync.dma_start(lwt_sb[:], lwt[:])
            owt_sb = kc.tile([128, 2, D], BF16)
            nc.sync.dma_start(owt_sb[:], owt[:])
            onesq_sb = kc.tile([128, 128], BF16)
            nc.sync.dma_start(onesq_sb[:], onesq[:])
            enc_sb = kc.tile([128, BS, 4, D], BF16)
            nc.sync.dma_start(enc_sb[:], enc_r[:])
            # dummy exp: pulls the act-table load off the step-0 critical path
            warm = kc.tile([1, 1], F32)
            nc.scalar.activation(warm[:], onesq_sb[0:1, 0:1], AF.Exp)
            if cfg["any_bias"]:
                bqc_sb = kc.tile([128, 2], F32)
                nc.sync.dma_start(bqc_sb[:], bqc[:])
                bkc_sb = kc.tile([128, 2], F32)
                nc.sync.dma_start(bkc_sb[:], bkc[:])
                brow_sb = kc.tile([1, 2 * H + 4 * H + D], BF16)
                nc.sync.dma_start(brow_sb[:], brow[:])
                ones_sb = kc.tile([1, BS], BF16)
                nc.sync.dma_start(ones_sb[:], ones[:])
            if cfg["enc_mask"]:
                emadd_sb = kc.tile([128, BS, 4], F32)
                nc.sync.dma_start(emadd_sb[:], emadd[:])
            if cfg["out_mask"]:
                bmr_sb = kc.tile([BS, NB], BF16)
                nc.sync.dma_start(bmr_sb[:], bmr[:])
                onest_sb = kc.tile([1, TT], BF16)
                nc.sync.dma_start(onest_sb[:], onest[:])

            seqqt_sb = kc.tile([128, 2, BS, L], BF16)
            m2t_sb = kc.tile([128, 2, BS, L], BF16)
            ket_sb = kc.tile([128, 2, TT, BS], BF16)
            linT_sb = kc.tile([128, 2, BS, TT], BF16)

            # ---- phase 0: seq_qT, m2T, keyT_enc ----
            with (
                tc.tile_pool(name="enctp", bufs=1) as ep,
                tc.tile_pool(name="p0ps", bufs=2, space="PSUM") as p0,
            ):
                enct_sb = enct_sb0
                for b in range(BS):
                    for c in range(2):
                        ps = p0.tile([128, 512], F32)
                        for k in range(2):
                            nc.tensor.matmul(
                                ps[:],
                                wqt_sb[:, k, c * 128:(c + 1) * 128],
                                enct_sb[:, k, b, :],
                                start=(k == 0), stop=(k == 1),
                            )
                        if cfg["any_bias"]:
                            nc.scalar.activation(
                                seqqt_sb[:, c, b, :], ps[:], AF.Identity,
                                bias=bqc_sb[:, c:c + 1],
                            )
                        else:
                            nc.vector.tensor_copy(seqqt_sb[:, c, b, :], ps[:])
                for c in range(2):
                    ps = p0.tile([128, 512], F32)
                    for k in range(2):
                        nc.tensor.matmul(
                            ps[:, 0:TT * BS],
                            wket_sb[:, k, c * 128:(c + 1) * 128],
                            tgtt_sb[:, k, :, :],
                            start=(k == 0), stop=(k == 1),
                        )
                    if cfg["any_bias"]:
                        nc.scalar.activation(
                            ket_sb[:, c, :, :], ps[:, 0:TT * BS], AF.Identity,
                            bias=bkc_sb[:, c:c + 1],
                        )
                    else:
                        nc.vector.tensor_copy(ket_sb[:, c, :, :], ps[:, 0:TT * BS])

            # phase-2-only tensors: DMA'd after the phase-0 loads so they
            # don't delay phase 0/1 startup
            cembt_sb = kc.tile([128, 2, NCOM], BF16)
            nc.sync.dma_start(cembt_sb[:], cembt[:])
            obt_sb = kc.tile([128, 2, BS, NB], BF16)
            nc.sync.dma_start(obt_sb[:], obt[:])

            # ---- phase 1: 127 sequential steps, column form ----
            nb_ = 1 if cfg["any_bias"] else 0
            with (
                tc.tile_pool(name="scps", bufs=2, space="PSUM") as scps,
                tc.tile_pool(name="gpsp", bufs=1, space="PSUM") as gpsp,
                tc.tile_pool(name="smps", bufs=1, space="PSUM") as smps,
                tc.tile_pool(name="misc", bufs=1, space="PSUM") as mps,
                tc.tile_pool(name="m2ps", bufs=1, space="PSUM") as pm,
                tc.tile_pool(name="work", bufs=3) as sbw,
            ):
                hT_cur = None

                def lin_mms(hT):
                    lps = mps.tile([128, 2, BS], F32, tag="lps")
                    for c in range(2):
                        for k in range(2):
                            nc.tensor.matmul(
                                lps[:, c, :],
                                owt_sb[:, k, c * 128:(c + 1) * 128],
                                hT[:, k, :],
                                start=(k == 0), stop=(k == 1 and nb_ == 0),
                            )
                        if nb_:
                            nc.tensor.matmul(
                                lps[:, c, :],
                                brow_sb[0:1, 5 * H + c * 128:5 * H + (c + 1) * 128],
                                ones_sb[0:1, :],
                                start=False, stop=True,
                            )
                    return lps

                def m2_mms(b):
                    # m2T[:, :, b, :] is first needed by step b+1's h-half;
                    # computing example b's slice at the end of step b keeps
                    # step 0 from waiting on it and spreads the copies out
                    for c in range(2):
                        ps = pm.tile([128, 512], F32, tag="m2p")
                        for k in range(2):
                            nc.tensor.matmul(
                                ps[:],
                                wkh2_sb[:, k, c * 128:(c + 1) * 128],
                                seqqt_sb[:, k, b, :],
                                start=(k == 0), stop=(k == 1),
                            )
                        eng = nc.vector if c == 0 else nc.scalar
                        if c == 0:
                            nc.vector.tensor_copy(m2t_sb[:, c, b, :], ps[:])
                        else:
                            nc.scalar.activation(
                                m2t_sb[:, c, b, :], ps[:], AF.Identity
                            )

                for t in range(0 if cfg.get("skip_p1") else TT):
                    # -- scores (psum col [b, lc]): enc half has no h dep --
                    # NOTE: one accumulation group for the whole tile per step
                    # (start only on the first mm, stop only on the last):
                    # interleaved per-column groups in one psum zero region
                    # corrupt each other (start re-marks the region pending-
                    # zero, so open columns lose their partial sums).
                    scp = scps.tile([128, BS, 4], F32, tag="scp")
                    for b in range(BS):
                        for lc in range(4):
                            col = scp[:, b, lc:lc + 1]
                            for k in range(2):
                                nc.tensor.matmul(
                                    col,
                                    seqqt_sb[:, k, b, lc * 128:(lc + 1) * 128],
                                    ket_sb[:, k, t, b:b + 1],
                                    start=(b == 0 and lc == 0 and k == 0),
                                    stop=(t == 0 and b == BS - 1 and lc == 3 and k == 1),
                                )
                    if t > 0:
                        for b in range(BS):
                            for lc in range(4):
                                col = scp[:, b, lc:lc + 1]
                                for k in range(2):
                                    nc.tensor.matmul(
                                        col,
                                        m2t_sb[:, k, b, lc * 128:(lc + 1) * 128],
                                        hT_cur[:, k, b:b + 1],
                                        start=False,
                                        stop=(b == BS - 1 and lc == 3 and k == 1),
                                    )
                    # gates h-half early (off critical path); single
                    # accumulation group per step (see scores note)
                    gp = gpsp.tile([128, 8, BS], F32, tag="gp")
                    if t > 0:
                        for g in range(8):
                            for k in range(2):
                                nc.tensor.matmul(
                                    gp[:, g, :],
                                    lwt_sb[:, 2 + k, g * 128:(g + 1) * 128],
                                    hT_cur[:, k, :],
                                    start=(g == 0 and k == 0), stop=False,
                                )
                        # output projection for step t-1 (h_{t-1} ready now)
                        lps = lin_mms(hT_cur)
                        nc.vector.tensor_copy(linT_sb[:, :, :, t - 1], lps[:])

                    if cfg["enc_mask"]:
                        nc.vector.tensor_add(scp[:], scp[:], emadd_sb[:])

                    # -- softmax: exp -> sums (4 accumulated ones-matmuls,
                    #    replicated over partitions) -> recip -> scale --
                    wt = sbw.tile([128, BS, 4], BF16, tag="wt")
                    nc.scalar.activation(wt[:], scp[:], AF.Exp)
                    smp = smps.tile([128, 1, BS], F32, tag="smp")
                    for lc in range(4):
                        nc.tensor.matmul(
                            smp[:, 0, :], onesq_sb[:], wt[:, :, lc],
                            start=(lc == 0), stop=(lc == 3),
                        )
                    # attention context, column form
                    atp = mps.tile([128, 2, BS], F32, tag="atp")
                    for b in range(BS):
                        for dc in range(2):
                            for lc in range(4):
                                nc.tensor.matmul(
                                    atp[:, dc, b:b + 1],
                                    enc_sb[:, b, lc, dc * 128:(dc + 1) * 128],
                                    wt[:, b, lc:lc + 1],
                                    start=(lc == 0), stop=(lc == 3),
                                )
                    rb = sbw.tile([128, 1, BS], F32, tag="rb")
                    nc.vector.reciprocal(rb[:, 0, :], smp[:, 0, :])
                    ats = sbw.tile([128, 2, BS], BF16, tag="ats")
                    nc.vector.tensor_mul(
                        ats[:], atp[:], rb[:].broadcast_to([128, 2, BS])
                    )

                    # -- combine + relu --
                    cbp = mps.tile([128, 2, BS], F32, tag="cbp")
                    cl = [tgtt_sb[:, 0, t, :], tgtt_sb[:, 1, t, :],
                          ats[:, 0, :], ats[:, 1, :]]
                    for fc in range(2):
                        for k in range(4):
                            nc.tensor.matmul(
                                cbp[:, fc, :],
                                cwt_sb[:, k, fc * 128:(fc + 1) * 128],
                                cl[k],
                                start=(k == 0), stop=(k == 3 and nb_ == 0),
                            )
                        if nb_:
                            nc.tensor.matmul(
                                cbp[:, fc, :],
                                brow_sb[0:1, fc * 128:(fc + 1) * 128],
                                ones_sb[0:1, :],
                                start=False, stop=True,
                            )
                    cbT = sbw.tile([128, 2, BS], BF16, tag="cbT")
                    nc.vector.tensor_scalar_max(cbT[:], cbp[:], 0.0)

                    # -- LSTM gates, comb half (accumulates into gp) --
                    for g in range(8):
                        for k in range(2):
                            nc.tensor.matmul(
                                gp[:, g, :],
                                lwt_sb[:, k, g * 128:(g + 1) * 128],
                                cbT[:, k, :],
                                start=(t == 0 and g == 0 and k == 0),
                                stop=(g == 7 and k == 1 and nb_ == 0),
                            )
                        if nb_:
                            nc.tensor.matmul(
                                gp[:, g, :],
                                brow_sb[0:1, 2 * H + g * 128:2 * H + (g + 1) * 128],
                                ones_sb[0:1, :],
                                start=False, stop=(g == 7),
                            )

                    # -- gate nonlinearities, one exp for all gates:
                    #    sigmoid(x) = 1/(1+exp(-x)); tanh(g) = 2*sigmoid(2g)-1
                    #    (g-gate weights are pre-doubled on the host) --
                    sie = sbw.tile([128, 8, BS], F32, tag="sie")
                    nc.scalar.activation(sie[:], gp[:], AF.Exp, scale=-1.0)
                    si = sbw.tile([128, 8, BS], F32, tag="si")
                    nc.scalar.activation(si[:], sie[:], AF.Identity, bias=1.0)
                    nc.vector.reciprocal(si[:], si[:])
                    # gc holds [tanh(g) | c_{t-1}] so si[i,f] multiplies both
                    # in a single op: m12 = [si_i*tg | si_f*c]
                    gc = gc_next if t > 0 else stp.tile([128, 4, BS], F32, tag="gc")
                    nc.vector.tensor_scalar(
                        gc[:, 0:2, :], si[:, 6:8, :], 2.0, -1.0,
                        mybir.AluOpType.mult, mybir.AluOpType.add,
                    )

                    # -- c/h update --
                    gc_next = stp.tile([128, 4, BS], F32, tag="gc")
                    if t > 0:
                        m12 = sbw.tile([128, 4, BS], F32, tag="m12")
                        nc.vector.tensor_mul(m12[:], si[:, 0:4, :], gc[:])
                        c_new = gc_next[:, 2:4, :]
                        nc.vector.tensor_add(c_new, m12[:, 0:2, :], m12[:, 2:4, :])
                    else:
                        c_new = gc_next[:, 2:4, :]
                        nc.vector.tensor_mul(c_new, si[:, 0:2, :], gc[:, 0:2, :])
                    tc_ = sbw.tile([128, 2, BS], F32, tag="tc")
                    nc.scalar.activation(tc_[:], c_new, AF.Tanh)
                    hT_new = stp.tile([128, 2, BS], BF16, tag="hstate")
                    nc.vector.tensor_mul(hT_new[:], si[:, 4:6, :], tc_[:])

                    if cfg.get("dbg_lin"):
                        nc.sync.dma_start(dr_dbg["dbg_h"][:, :, :, t], hT_new[:])
                        nc.sync.dma_start(dr_dbg["dbg_c"][:, :, :, t], c_new[:])
                        nc.sync.dma_start(dr_dbg["dbg_wt"][:, :, :, t], wt[:])
                        scpc = sbw.tile([128, BS, 4], F32, tag="scpdbg")
                        nc.vector.tensor_copy(scpc[:], scp[:])
                        nc.sync.dma_start(dr_dbg["dbg_scp"][:, :, :, t], scpc[:])

                    hT_cur = hT_new
                    if t == 0:
                        for b_ in range(BS):
                            m2_mms(b_)

                if not cfg.get("skip_p1"):
                    lps = lin_mms(hT_cur)
                    nc.vector.tensor_copy(linT_sb[:, :, :, TT - 1], lps[:])

            if cfg.get("dbg_lin"):
                nc.sync.dma_start(dbglin[:], linT_sb[:])
                nc.sync.dma_start(dr_dbg["dbg_ket"][:], ket_sb[:])
                nc.sync.dma_start(dr_dbg["dbg_seqq"][:], seqqt_sb[:])
                nc.sync.dma_start(dr_dbg["dbg_m2"][:], m2t_sb[:])
            # ---- phase 2: vocab projection + log_softmax ----
            with (
                tc.tile_pool(name="p2ps", bufs=2, space="PSUM") as p2,
                tc.tile_pool(name="p2ps_b", bufs=2, space="PSUM") as p2b,
                tc.tile_pool(name="ep2", bufs=1) as ep2,
                tc.tile_pool(name="outst", bufs=3) as osp,
                tc.tile_pool(name="sm2", bufs=2) as sm2,
            ):
                # groups of vocab units paired into (TT, 1024) psum tiles
                # spanning two banks, to amortize Act/DVE per-op overheads:
                # 7x [512|512] common, 1x [512|320] common, 1x [128] batched
                P2G = []
                for i in range(7):
                    P2G.append((i * 1024, [(0, 512, 1024 * i), (512, 512, 1024 * i + 512)]))
                P2G.append((7168, [(0, 512, 7168), (512, 320, 7680)]))
                P2G.append((NCOM, [(0, NB, None)]))
                ngr = len(P2G)

                def group_mms(b, g, ps):
                    voff, units = P2G[g]
                    wtot = 0
                    for (boff, w, coff) in units:
                        if coff is not None:
                            rhs = [cembt_sb[:, k, coff:coff + w] for k in range(2)]
                            masked = False
                        else:
                            rhs = [obt_sb[:, k, b, :] for k in range(2)]
                            masked = cfg["out_mask"]
                        for k in range(2):
                            nc.tensor.matmul(
                                ps[:, boff:boff + w],
                                linT_sb[:, k, b, :],
                                rhs[k],
                                start=(k == 0),
                                stop=(k == 1 and not masked),
                            )
                        if masked:
                            nc.tensor.matmul(
                                ps[:, boff:boff + w], onest_sb[0:1, :],
                                bmr_sb[b:b + 1, :],
                                start=False, stop=True,
                            )
                        wtot = boff + w
                    return wtot

                # pass 1: exp-sums only (Act); pass 2 recomputes the cheap
                # matmul and writes out = logits - log(sumexp) on DVE, so the
                # expensive Ln pass over the full vocab disappears.
                for b in range(0 if cfg.get("skip_p2") else BS):
                    ss = sm2.tile([TT, ngr], F32, tag="ss")
                    for g in range(ngr):
                        ps = p2.tile([TT, 1024], F32, tag="p2")
                        w = group_mms(b, g, ps)
                        exs = ep2.tile([TT, 1024], BF16, tag="exs", bufs=3)
                        nc.scalar.activation(
                            exs[:, 0:w], ps[:, 0:w], AF.Exp,
                            accum_out=ss[:, g:g + 1],
                        )
                    st = sm2.tile([TT, 1], F32, tag="st")
                    nc.vector.reduce_sum(
                        st[:], ss[:], axis=mybir.AxisListType.X
                    )
                    lz = sm2.tile([TT, 1], F32, tag="lz")
                    nc.scalar.activation(lz[:], st[:], AF.Ln)
                    nlz = sm2.tile([TT, 1], F32, tag="nlz")
                    nc.vector.tensor_scalar_mul(nlz[:], lz[:], -1.0)
                    for g in range(ngr):
                        voff = P2G[g][0]
                        ps = p2b.tile([TT, 1024], F32, tag="p2b")
                        w = group_mms(b, g, ps)
                        ot = osp.tile([TT, 1024], F32, tag="ot", bufs=4)
                        if b == BS - 1 and g % 2 == 1:
                            # last example has no next-example exp stream to
                            # overlap with: split subs between DVE and Act
                            nc.scalar.activation(
                                ot[:, 0:w], ps[:, 0:w], AF.Identity, bias=nlz[:],
                            )
                        else:
                            nc.vector.tensor_scalar_sub(ot[:, 0:w], ps[:, 0:w], lz[:])
                        nc.sync.dma_start(
                            out[b, :, voff:voff + w], ot[:, 0:w]
                        )

    nc.compile()
    return nc


_CACHE = {}


def kernel(**inputs):
    inp = {k: np.asarray(v) for k, v in inputs.items()}
    enc = inp["encoder_outputs"].astype(np.float32)
    encm = inp["encoder_outputs_mask"]
    ob = inp["output_batched_encodings"].astype(np.float32)
    obm = inp["output_batched_encodings_mask"]
    idx = inp["target_idxs"]
    cem = inp["common_embedding"].astype(np.float32)
    akw = inp["attn_key_w"].astype(np.float32)
    akb = inp["attn_key_b"].astype(np.float32)
    aqw = inp["attn_query_w"].astype(np.float32)
    aqb = inp["attn_query_b"].astype(np.float32)
    cw = inp["combine_w"].astype(np.float32)
    cb = inp["combine_b"].astype(np.float32)
    wih = inp["lstm_w_ih"].astype(np.float32)
    whh = inp["lstm_w_hh"].astype(np.float32)
    bih = inp["lstm_b_ih"].astype(np.float32)
    bhh = inp["lstm_b_hh"].astype(np.float32)
    ow = inp["out_w"].astype(np.float32)
    obias = inp["out_b"].astype(np.float32)

    # teacher-forced embedding gather (host: data-dependent indexing)
    is_c = idx < NCOM
    cidx = np.clip(idx, 0, NCOM - 1)
    bidx = np.clip(idx - NCOM, 0, NB - 1)
    ge_c = cem[cidx]                                   # (B, T, D)
    ge_b = np.take_along_axis(ob, bidx[..., None], axis=1)
    tgt = np.where(is_c[..., None], ge_c, ge_b)[:, :TT, :].astype(np.float32)

    any_bias = bool(
        np.any(akb) or np.any(aqb) or np.any(cb) or np.any(bih)
        or np.any(bhh) or np.any(obias)
    )
    enc_mask = not bool(encm.all())
    out_mask = not bool(obm.all())

    cfg = {"any_bias": any_bias, "enc_mask": enc_mask, "out_mask": out_mask}
    key = (any_bias, enc_mask, out_mask)
    if key not in _CACHE:
        _CACHE[key] = _build(cfg)
    nc = _CACHE[key]

    # lstm weights, gate order [i, f, o, g]; g doubled so one exp computes all
    # gates: tanh(g) = 2*sigmoid(2g)-1
    lcat = np.concatenate([wih.T, whh.T], axis=0)       # (2*H(g), 4*H)
    perm = np.concatenate(
        [lcat[:, 0:H], lcat[:, H:2 * H], lcat[:, 3 * H:4 * H],
         2.0 * lcat[:, 2 * H:3 * H]],
        axis=1,
    )

    # shared (replicated) tensors
    import ml_dtypes
    bft = ml_dtypes.bfloat16
    shared = {
        "cembt": _pmajor(np.ascontiguousarray(cem.T), 2).astype(bft),
        "wqt": _pmajor(np.ascontiguousarray(aqw.T), 2).astype(bft),
        "wket": _pmajor(np.ascontiguousarray(akw[:, :D].T), 2).astype(bft),
        "wkh2": _pmajor(np.ascontiguousarray(akw[:, D:]), 2).astype(bft),
        "cwt": _pmajor(np.ascontiguousarray(cw.T), 4).astype(bft),
        "lwt": _pmajor(np.ascontiguousarray(perm), 4).astype(bft),
        "owt": _pmajor(np.ascontiguousarray(ow.T), 2).astype(bft),
        "onesq": np.ones((128, 128), bft),
    }
    if any_bias:
        bl = bih + bhh
        blp = np.concatenate(
            [bl[0:H], bl[H:2 * H], bl[3 * H:4 * H], 2.0 * bl[2 * H:3 * H]]
        )
        shared["bqc"] = _pmajor(aqb, 2)
        shared["bkc"] = _pmajor(akb, 2)
        shared["brow"] = np.concatenate(
            [cb, blp, obias]
        )[None, :].astype(bft)
        shared["ones"] = np.ones((1, BS), bft)
    if out_mask:
        shared["onest"] = np.ones((1, TT), bft)

    in_maps = []
    for c in range(NCORES):
        sl = slice(c * BS, (c + 1) * BS)
        e = enc[sl]                                    # (BS, L, D)
        tg_ = tgt[sl]                                  # (BS, TT, D)
        obs = ob[sl]                                   # (BS, NB, D)
        m = dict(shared)
        m["enc_r"] = np.ascontiguousarray(
            e.reshape(BS, 4, 128, D).transpose(2, 0, 1, 3)
        ).astype(bft)
        m["enct"] = np.ascontiguousarray(
            e.transpose(2, 0, 1).reshape(2, 128, BS, L).transpose(1, 0, 2, 3)
        ).astype(bft)
        m["tgtt"] = np.ascontiguousarray(
            tg_.transpose(2, 1, 0).reshape(2, 128, TT, BS).transpose(1, 0, 2, 3)
        ).astype(bft)
        m["obt"] = np.ascontiguousarray(
            obs.transpose(2, 0, 1).reshape(2, 128, BS, NB).transpose(1, 0, 2, 3)
        ).astype(ml_dtypes.bfloat16)
        if enc_mask:
            em = np.where(encm[sl], 0.0, -1e30).astype(np.float32)  # (BS, L)
            m["emadd"] = np.ascontiguousarray(
                em.reshape(BS, 4, 128).transpose(2, 0, 1)
            )
        if out_mask:
            m["bmr"] = np.where(obm[sl], 0.0, -1e30).astype(bft)
        in_maps.append(m)

    res = run_bass_kernel_spmd(nc, in_maps, list(range(NCORES)))
    outs = [res.results[c]["out"].reshape(BS, TT, V) for c in range(NCORES)]
    return np.concatenate(outs, axis=0).astype(np.float32)


# revision 30
# speedup vs baseline: 1.0089x; 1.0057x over previous
"""AttnRNN decoder kernel for trn2 (8 NeuronCores, data-parallel over batch).

Structure:
  host   : embedding gather, weight transposes, batch sharding (B=32 -> 4/core)
  device : phase0  seq_qT / m2T / keyT_enc precompute (t-independent terms)
           phase1  127 sequential steps, fully column-form (features on
                   partitions, batch=4 moving dim; no transposes; sigmoid via
                   exp+reciprocal so only one act-table set is used)
           phase2  hoisted vocab projection (bf16) + log_softmax + DMA out
"""

import numpy as np

import concourse.bass as bass
import concourse.bacc as bacc
import concourse.mybir as mybir
import concourse.tile as tile
from concourse.bass_utils import run_bass_kernel_spmd

F32 = mybir.dt.float32
BF16 = mybir.dt.bfloat16
AF = mybir.ActivationFunctionType

B, L, D, T = 32, 512, 256, 128
H, NCOM, NB = 256, 8000, 128
V = NCOM + NB
NCORES = 8
BS = B // NCORES          # 4 examples per core
TT = T - 1                # 127 decode steps

# vocab chunking for phase 2: 15x512 + 1x320 common, then 128 batched
VCH = [(i * 512, 512) for i in range(15)] + [(7680, 320)]


def _pmajor(x, nchunk):
    """(nchunk*128, ...) -> (128, nchunk, ...) partition-major."""
    s = x.shape
    return np.ascontiguousarray(
        x.reshape(nchunk, 128, *s[1:]).transpose(1, 0, *range(2, 1 + len(s)))
    )


def _build(cfg):
    """Build the single-core program (SPMD-replicated across 8 cores)."""
    nc = bacc.Bacc("TRN2", target_bir_lowering=False, debug=False)

    dr = {}

    def din(name, shape, dt=F32):
        dr[name] = nc.dram_tensor(name, list(shape), dt, kind="ExternalInput").ap()
        return dr[name]

    enc_r = din("enc_r", (128, BS, 4, D), BF16)    # enc[b, 128*lc+p, d]
    enct = din("enct", (128, 2, BS, L), BF16)      # enc[b, l, 128*dc+p]
    tgtt = din("tgtt", (128, 2, TT, BS), BF16)     # tgt[b, t, 128*dc+p]
    cembt = din("cembt", (128, 2, NCOM), BF16)     # common[v, 128*dc+p]
    obt = din("obt", (128, 2, BS, NB), BF16)       # batched[b, v, 128*dc+p]
    wqt = din("wqt", (128, 2, D), BF16)                  # Wq[e, 128*dc+p]
    wket = din("wket", (128, 2, D), BF16)
    wkh2 = din("wkh2", (128, 2, D), BF16)   # akw[e, 256+f] with e on partitions
    cwt = din("cwt", (128, 4, H), BF16)                  # combine_w[g, 128*fc+p]
    lwt = din("lwt", (128, 4, 4 * H), BF16)              # [W_ih.T ; W_hh.T], ifog->ifog perm
    owt = din("owt", (128, 2, D), BF16)
    onesq = din("onesq", (128, 128), BF16)
    if cfg["any_bias"]:
        bqc = din("bqc", (128, 2))
        bkc = din("bkc", (128, 2))
        brow = din("brow", (1, 2 * H + 4 * H + D), BF16)   # [bc(256), bl(1024, ifog-perm), bo(256)]
        ones = din("ones", (1, BS), BF16)
    if cfg["enc_mask"]:
        emadd = din("emadd", (128, BS, 4))
    if cfg["out_mask"]:
        bmr = din("bmr", (BS, NB), BF16)
        onest = din("onest", (1, TT), BF16)

    out = nc.dram_tensor("out", [BS, TT, V], F32, kind="ExternalOutput").ap()
    if cfg.get("dbg_lin"):
        dbglin = nc.dram_tensor("dbglin", [128, 2, BS, TT], BF16, kind="ExternalOutput").ap()
        dr_dbg = {}
        for nm_, sh_, dt_ in [("dbg_wt", [128, BS, 4, TT], BF16),
                              ("dbg_c", [128, 2, BS, TT], F32),
                              ("dbg_h", [128, 2, BS, TT], BF16),
                              ("dbg_scp", [128, BS, 4, TT], F32),
                              ("dbg_ket", [128, 2, TT, BS], BF16),
                              ("dbg_seqq", [128, 2, BS, L], BF16),
                              ("dbg_m2", [128, 2, BS, L], BF16)]:
            dr_dbg[nm_] = nc.dram_tensor(nm_, sh_, dt_, kind="ExternalOutput").ap()

    with tile.TileContext(nc) as tc:
        with (
            tc.tile_pool(name="const", bufs=1) as kc,
            tc.tile_pool(name="state", bufs=3) as stp,
        ):
            # ---- persistent SBUF loads (ordered to unblock phase 0/1) ----
            enct_sb0 = kc.tile([128, 2, BS, L], BF16)
            nc.sync.dma_start(enct_sb0[:], enct[:])
            wqt_sb = kc.tile([128, 2, D], BF16)
            nc.sync.dma_start(wqt_sb[:], wqt[:])
            wket_sb = kc.tile([128, 2, D], BF16)
            nc.sync.dma_start(wket_sb[:], wket[:])
            tgtt_sb = kc.tile([128, 2, TT, BS], BF16)
            nc.sync.dma_start(tgtt_sb[:], tgtt[:])
            wkh2_sb = kc.tile([128, 2, D], BF16)
            nc.sync.dma_start(wkh2_sb[:], wkh2[:])
            cwt_sb = kc.tile([128, 4, H], BF16)
            nc.sync.dma_start(cwt_sb[:], cwt[:])
            lwt_sb = kc.tile([128, 4, 4 * H], BF16)
            nc.sync.dma_start(lwt_sb[:], lwt[:])
            owt_sb = kc.tile([128, 2, D], BF16)
            nc.sync.dma_start(owt_sb[:], owt[:])
            onesq_sb = kc.tile([128, 128], BF16)
            nc.sync.dma_start(onesq_sb[:], onesq[:])
            enc_sb = kc.tile([128, BS, 4, D], BF16)
            nc.sync.dma_start(enc_sb[:], enc_r[:])
            # dummy exp: pulls the act-table load off the step-0 critical path
            warm = kc.tile([1, 1], F32)
            nc.scalar.activation(warm[:], onesq_sb[0:1, 0:1], AF.Exp)
            if cfg["any_bias"]:
                bqc_sb = kc.tile([128, 2], F32)
                nc.sync.dma_start(bqc_sb[:], bqc[:])
                bkc_sb = kc.tile([128, 2], F32)
                nc.sync.dma_start(bkc_sb[:], bkc[:])
                brow_sb = kc.tile([1, 2 * H + 4 * H + D], BF16)
                nc.sync.dma_start(brow_sb[:], brow[:])
                ones_sb = kc.tile([1, BS], BF16)
                nc.sync.dma_start(ones_sb[:], ones[:])
            if cfg["enc_mask"]:
                emadd_sb = kc.tile([128, BS, 4], F32)
                nc.sync.dma_start(emadd_sb[:], emadd[:])
            if cfg["out_mask"]:
                bmr_sb = kc.tile([BS, NB], BF16)
                nc.sync.dma_start(bmr_sb[:], bmr[:])
                onest_sb = kc.tile([1, TT], BF16)
                nc.sync.dma_start(onest_sb[:], onest[:])

            seqqt_sb = kc.tile([128, 2, BS, L], BF16)
            m2t_sb = kc.tile([128, 2, BS, L], BF16)
            ket_sb = kc.tile([128, 2, TT, BS], BF16)
            linT_sb = kc.tile([128, 2, BS, TT], BF16)

            # ---- phase 0: seq_qT, m2T, keyT_enc ----
            with (
                tc.tile_pool(name="enctp", bufs=1) as ep,
                tc.tile_pool(name="p0ps", bufs=2, space="PSUM") as p0,
            ):
                enct_sb = enct_sb0
                for b in range(BS):
                    for c in range(2):
                        ps = p0.tile([128, 512], F32)
                        for k in range(2):
                            nc.tensor.matmul(
                                ps[:],
                                wqt_sb[:, k, c * 128:(c + 1) * 128],
                                enct_sb[:, k, b, :],
                                start=(k == 0), stop=(k == 1),
                            )
                        if cfg["any_bias"]:
                            nc.scalar.activation(
                                seqqt_sb[:, c, b, :], ps[:], AF.Identity,
                                bias=bqc_sb[:, c:c + 1],
                            )
                        else:
                            nc.vector.tensor_copy(seqqt_sb[:, c, b, :], ps[:])
                for c in range(2):
                    ps = p0.tile([128, 512], F32)
                    for k in range(2):
                        nc.tensor.matmul(
                            ps[:, 0:TT * BS],
                            wket_sb[:, k, c * 128:(c + 1) * 128],
                            tgtt_sb[:, k, :, :],
                            start=(k == 0), stop=(k == 1),
                        )
                    if cfg["any_bias"]:
                        nc.scalar.activation(
                            ket_sb[:, c, :, :], ps[:, 0:TT * BS], AF.Identity,
                            bias=bkc_sb[:, c:c + 1],
                        )
                    else:
                        nc.vector.tensor_copy(ket_sb[:, c, :, :], ps[:, 0:TT * BS])

            # phase-2-only tensors: DMA'd after the phase-0 loads so they
            # don't delay phase 0/1 startup
            cembt_sb = kc.tile([128, 2, NCOM], BF16)
            nc.sync.dma_start(cembt_sb[:], cembt[:])
            obt_sb = kc.tile([128, 2, BS, NB], BF16)
            nc.sync.dma_start(obt_sb[:], obt[:])

            # ---- phase 1: 127 sequential steps, column form ----
            nb_ = 1 if cfg["any_bias"] else 0
            with (
                tc.tile_pool(name="scps", bufs=2, space="PSUM") as scps,
                tc.tile_pool(name="gpsp", bufs=1, space="PSUM") as gpsp,
                tc.tile_pool(name="smps", bufs=1, space="PSUM") as smps,
                tc.tile_pool(name="misc", bufs=1, space="PSUM") as mps,
                tc.tile_pool(name="m2ps", bufs=1, space="PSUM") as pm,
                tc.tile_pool(name="work", bufs=3) as sbw,
            ):
                hT_cur = None

                def lin_mms(hT):
                    lps = mps.tile([128, 2, BS], F32, tag="lps")
                    for c in range(2):
                        for k in range(2):
                            nc.tensor.matmul(
                                lps[:, c, :],
                                owt_sb[:, k, c * 128:(c + 1) * 128],
                                hT[:, k, :],
                                start=(k == 0), stop=(k == 1 and nb_ == 0),
                            )
                        if nb_:
                            nc.tensor.matmul(
                                lps[:, c, :],
                                brow_sb[0:1, 5 * H + c * 128:5 * H + (c + 1) * 128],
                                ones_sb[0:1, :],
                                start=False, stop=True,
                            )
                    return lps

                def m2_mms(b):
                    # m2T[:, :, b, :] is first needed by step b+1's h-half;
                    # computing example b's slice at the end of step b keeps
                    # step 0 from waiting on it and spreads the copies out
                    for c in range(2):
                        ps = pm.tile([128, 512], F32, tag="m2p")
                        for k in range(2):
                            nc.tensor.matmul(
                                ps[:],
                                wkh2_sb[:, k, c * 128:(c + 1) * 128],
                                seqqt_sb[:, k, b, :],
                                start=(k == 0), stop=(k == 1),
                            )
                        eng = nc.vector if c == 0 else nc.scalar
                        if c == 0:
                            nc.vector.tensor_copy(m2t_sb[:, c, b, :], ps[:])
                        else:
                            nc.scalar.activation(
                                m2t_sb[:, c, b, :], ps[:], AF.Identity
                            )

                for t in range(0 if cfg.get("skip_p1") else TT):
                    # -- scores (psum col [b, lc]): enc half has no h dep --
                    # NOTE: one accumulation group for the whole tile per step
                    # (start only on the first mm, stop only on the last):
                    # interleaved per-column groups in one psum zero region
                    # corrupt each other (start re-marks the region pending-
                    # zero, so open columns lose their partial sums).
                    scp = scps.tile([128, BS, 4], F32, tag="scp")
                    for b in range(BS):
                        for lc in range(4):
                            col = scp[:, b, lc:lc + 1]
                            for k in range(2):
                                nc.tensor.matmul(
                                    col,
                                    seqqt_sb[:, k, b, lc * 128:(lc + 1) * 128],
                                    ket_sb[:, k, t, b:b + 1],
                                    start=(b == 0 and lc == 0 and k == 0),
                                    stop=(t == 0 and b == BS - 1 and lc == 3 and k == 1),
                                )
                    if t > 0:
                        for b in range(BS):
                            for lc in range(4):
                                col = scp[:, b, lc:lc + 1]
                                for k in range(2):
                                    nc.tensor.matmul(
                                        col,
                                        m2t_sb[:, k, b, lc * 128:(lc + 1) * 128],
                                        hT_cur[:, k, b:b + 1],
                                        start=False,
                                        stop=(b == BS - 1 and lc == 3 and k == 1),
                                    )
                    # gates h-half early (off critical path); single
                    # accumulation group per step (see scores note)
                    gp = gpsp.tile([128, 8, BS], F32, tag="gp")
                    if t > 0:
                        for g in range(8):
                            for k in range(2):
                                nc.tensor.matmul(
                                    gp[:, g, :],
                                    lwt_sb[:, 2 + k, g * 128:(g + 1) * 128],
                                    hT_cur[:, k, :],
                                    start=(g == 0 and k == 0), stop=False,
                                )
                        # output projection for step t-1 (h_{t-1} ready now)
                        lps = lin_mms(hT_cur)
                        nc.vector.tensor_copy(linT_sb[:, :, :, t - 1], lps[:])

                    if cfg["enc_mask"]:
                        nc.vector.tensor_add(scp[:], scp[:], emadd_sb[:])

                    # -- softmax: exp -> sums (4 accumulated ones-matmuls,
                    #    replicated over partitions) -> recip -> scale --
                    wt = sbw.tile([128, BS, 4], BF16, tag="wt")
                    nc.scalar.activation(wt[:], scp[:], AF.Exp)
                    smp = smps.tile([128, 1, BS], F32, tag="smp")
                    for lc in range(4):
                        nc.tensor.matmul(
                            smp[:, 0, :], onesq_sb[:], wt[:, :, lc],
                            start=(lc == 0), stop=(lc == 3),
                        )
                    # attention context, column form
                    atp = mps.tile([128, 2, BS], F32, tag="atp")
                    for b in range(BS):
                        for dc in range(2):
                            for lc in range(4):
                                nc.tensor.matmul(
                                    atp[:, dc, b:b + 1],
                                    enc_sb[:, b, lc, dc * 128:(dc + 1) * 128],
                                    wt[:, b, lc:lc + 1],
                                    start=(lc == 0), stop=(lc == 3),
                                )
                    rb = sbw.tile([128, 1, BS], F32, tag="rb")
                    nc.vector.reciprocal(rb[:, 0, :], smp[:, 0, :])
                    ats = sbw.tile([128, 2, BS], BF16, tag="ats")
                    nc.vector.tensor_mul(
                        ats[:], atp[:], rb[:].broadcast_to([128, 2, BS])
                    )

                    # -- combine + relu --
                    cbp = mps.tile([128, 2, BS], F32, tag="cbp")
                    cl = [tgtt_sb[:, 0, t, :], tgtt_sb[:, 1, t, :],
                          ats[:, 0, :], ats[:, 1, :]]
                    for fc in range(2):
                        for k in range(4):
                            nc.tensor.matmul(
                                cbp[:, fc, :],
                                cwt_sb[:, k, fc * 128:(fc + 1) * 128],
                                cl[k],
                                start=(k == 0), stop=(k == 3 and nb_ == 0),
                            )
                        if nb_:
                            nc.tensor.matmul(
                                cbp[:, fc, :],
                                brow_sb[0:1, fc * 128:(fc + 1) * 128],
                                ones_sb[0:1, :],
                                start=False, stop=True,
                            )
                    cbT = sbw.tile([128, 2, BS], BF16, tag="cbT")
                    nc.vector.tensor_scalar_max(cbT[:], cbp[:], 0.0)

                    # -- LSTM gates, comb half (accumulates into gp) --
                    for g in range(8):
                        for k in range(2):
                            nc.tensor.matmul(
                                gp[:, g, :],
                                lwt_sb[:, k, g * 128:(g + 1) * 128],
                                cbT[:, k, :],
                                start=(t == 0 and g == 0 and k == 0),
                                stop=(g == 7 and k == 1 and nb_ == 0),
                            )
                        if nb_:
                            nc.tensor.matmul(
                                gp[:, g, :],
                                brow_sb[0:1, 2 * H + g * 128:2 * H + (g + 1) * 128],
                                ones_sb[0:1, :],
                                start=False, stop=(g == 7),
                            )

                    # -- gate nonlinearities, one exp for all gates:
                    #    sigmoid(x) = 1/(1+exp(-x)); tanh(g) = 2*sigmoid(2g)-1
                    #    (g-gate weights are pre-doubled on the host) --
                    sie = sbw.tile([128, 8, BS], F32, tag="sie")
                    nc.scalar.activation(sie[:], gp[:], AF.Exp, scale=-1.0)
                    si = sbw.tile([128, 8, BS], F32, tag="si")
                    nc.vector.tensor_scalar_add(si[:], sie[:], 1.0)
                    nc.vector.reciprocal(si[:], si[:])
                    # gc holds [tanh(g) | c_{t-1}] so si[i,f] multiplies both
                    # in a single op: m12 = [si_i*tg | si_f*c]
                    gc = gc_next if t > 0 else stp.tile([128, 4, BS], F32, tag="gc")
                    nc.vector.tensor_scalar(
                        gc[:, 0:2, :], si[:, 6:8, :], 2.0, -1.0,
                        mybir.AluOpType.mult, mybir.AluOpType.add,
                    )

                    # -- c/h update --
                    gc_next = stp.tile([128, 4, BS], F32, tag="gc")
                    if t > 0:
                        m12 = sbw.tile([128, 4, BS], F32, tag="m12")
                        nc.vector.tensor_mul(m12[:], si[:, 0:4, :], gc[:])
                        c_new = gc_next[:, 2:4, :]
                        nc.vector.tensor_add(c_new, m12[:, 0:2, :], m12[:, 2:4, :])
                    else:
                        c_new = gc_next[:, 2:4, :]
                        nc.vector.tensor_mul(c_new, si[:, 0:2, :], gc[:, 0:2, :])
                    tc_ = sbw.tile([128, 2, BS], F32, tag="tc")
                    nc.scalar.activation(tc_[:], c_new, AF.Tanh)
                    hT_new = stp.tile([128, 2, BS], BF16, tag="hstate")
                    nc.vector.tensor_mul(hT_new[:], si[:, 4:6, :], tc_[:])

                    if cfg.get("dbg_lin"):
                        nc.sync.dma_start(dr_dbg["dbg_h"][:, :, :, t], hT_new[:])
                        nc.sync.dma_start(dr_dbg["dbg_c"][:, :, :, t], c_new[:])
                        nc.sync.dma_start(dr_dbg["dbg_wt"][:, :, :, t], wt[:])
                        scpc = sbw.tile([128, BS, 4], F32, tag="scpdbg")
                        nc.vector.tensor_copy(scpc[:], scp[:])
                        nc.sync.dma_start(dr_dbg["dbg_scp"][:, :, :, t], scpc[:])

                    hT_cur = hT_new
                    if t == 0:
                        for b_ in range(BS):
                            m2_mms(b_)

                if not cfg.get("skip_p1"):
                    lps = lin_mms(hT_cur)
                    nc.vector.tensor_copy(linT_sb[:, :, :, TT - 1], lps[:])

            if cfg.get("dbg_lin"):
                nc.sync.dma_start(dbglin[:], linT_sb[:])
                nc.sync.dma_start(dr_dbg["dbg_ket"][:], ket_sb[:])
                nc.sync.dma_start(dr_dbg["dbg_seqq"][:], seqqt_sb[:])
                nc.sync.dma_start(dr_dbg["dbg_m2"][:], m2t_sb[:])
            # ---- phase 2: vocab projection + log_softmax ----
            with (
                tc.tile_pool(name="p2ps", bufs=2, space="PSUM") as p2,
                tc.tile_pool(name="p2ps_b", bufs=2, space="PSUM") as p2b,
                tc.tile_pool(name="ep2", bufs=1) as ep2,
                tc.tile_pool(name="outst", bufs=3) as osp,
                tc.tile_pool(name="sm2", bufs=2) as sm2,
            ):
                # groups of vocab units paired into (TT, 1024) psum tiles
                # spanning two banks, to amortize Act/DVE per-op overheads:
                # 7x [512|512] common, 1x [512|320] common, 1x [128] batched
                P2G = []
                for i in range(7):
                    P2G.append((i * 1024, [(0, 512, 1024 * i), (512, 512, 1024 * i + 512)]))
                P2G.append((7168, [(0, 512, 7168), (512, 320, 7680)]))
                P2G.append((NCOM, [(0, NB, None)]))
                ngr = len(P2G)

                def group_mms(b, g, ps):
                    voff, units = P2G[g]
                    wtot = 0
                    for (boff, w, coff) in units:
                        if coff is not None:
                            rhs = [cembt_sb[:, k, coff:coff + w] for k in range(2)]
                            masked = False
                        else:
                            rhs = [obt_sb[:, k, b, :] for k in range(2)]
                            masked = cfg["out_mask"]
                        for k in range(2):
                            nc.tensor.matmul(
                                ps[:, boff:boff + w],
                                linT_sb[:, k, b, :],
                                rhs[k],
                                start=(k == 0),
                                stop=(k == 1 and not masked),
                            )
                        if masked:
                            nc.tensor.matmul(
                                ps[:, boff:boff + w], onest_sb[0:1, :],
                                bmr_sb[b:b + 1, :],
                                start=False, stop=True,
                            )
                        wtot = boff + w
                    return wtot

                # pass 1: exp-sums only (Act); pass 2 recomputes the cheap
                # matmul and writes out = logits - log(sumexp) on DVE, so the
                # expensive Ln pass over the full vocab disappears.
                for b in range(0 if cfg.get("skip_p2") else BS):
                    ss = sm2.tile([TT, ngr], F32, tag="ss")
                    for g in range(ngr):
                        ps = p2.tile([TT, 1024], F32, tag="p2")
                        w = group_mms(b, g, ps)
                        exs = ep2.tile([TT, 1024], BF16, tag="exs", bufs=3)
                        nc.scalar.activation(
                            exs[:, 0:w], ps[:, 0:w], AF.Exp,
                            accum_out=ss[:, g:g + 1],
                        )
                    st = sm2.tile([TT, 1], F32, tag="st")
                    nc.vector.reduce_sum(
                        st[:], ss[:], axis=mybir.AxisListType.X
                    )
                    lz = sm2.tile([TT, 1], F32, tag="lz")
                    nc.scalar.activation(lz[:], st[:], AF.Ln)
                    nlz = sm2.tile([TT, 1], F32, tag="nlz")
                    nc.vector.tensor_scalar_mul(nlz[:], lz[:], -1.0)
                    for g in range(ngr):
                        voff = P2G[g][0]
                        ps = p2b.tile([TT, 1024], F32, tag="p2b")
                        w = group_mms(b, g, ps)
                        ot = osp.tile([TT, 1024], F32, tag="ot", bufs=4)
                        if b == BS - 1 and g % 2 == 1:
                            # last example has no next-example exp stream to
                            # overlap with: split subs between DVE and Act
                            nc.scalar.activation(
                                ot[:, 0:w], ps[:, 0:w], AF.Identity, bias=nlz[:],
                            )
                        else:
                            nc.vector.tensor_scalar_sub(ot[:, 0:w], ps[:, 0:w], lz[:])
                        nc.sync.dma_start(
                            out[b, :, voff:voff + w], ot[:, 0:w]
                        )

    nc.compile()
    return nc


_CACHE = {}


def kernel(**inputs):
    inp = {k: np.asarray(v) for k, v in inputs.items()}
    enc = inp["encoder_outputs"].astype(np.float32)
    encm = inp["encoder_outputs_mask"]
    ob = inp["output_batched_encodings"].astype(np.float32)
    obm = inp["output_batched_encodings_mask"]
    idx = inp["target_idxs"]
    cem = inp["common_embedding"].astype(np.float32)
    akw = inp["attn_key_w"].astype(np.float32)
    akb = inp["attn_key_b"].astype(np.float32)
    aqw = inp["attn_query_w"].astype(np.float32)
    aqb = inp["attn_query_b"].astype(np.float32)
    cw = inp["combine_w"].astype(np.float32)
    cb = inp["combine_b"].astype(np.float32)
    wih = inp["lstm_w_ih"].astype(np.float32)
    whh = inp["lstm_w_hh"].astype(np.float32)
    bih = inp["lstm_b_ih"].astype(np.float32)
    bhh = inp["lstm_b_hh"].astype(np.float32)
    ow = inp["out_w"].astype(np.float32)
    obias = inp["out_b"].astype(np.float32)

    # teacher-forced embedding gather (host: data-dependent indexing)
    is_c = idx < NCOM
    cidx = np.clip(idx, 0, NCOM - 1)
    bidx = np.clip(idx - NCOM, 0, NB - 1)
    ge_c = cem[cidx]                                   # (B, T, D)
    ge_b = np.take_along_axis(ob, bidx[..., None], axis=1)
    tgt = np.where(is_c[..., None], ge_c, ge_b)[:, :TT, :].astype(np.float32)

    any_bias = bool(
        np.any(akb) or np.any(aqb) or np.any(cb) or np.any(bih)
        or np.any(bhh) or np.any(obias)
    )
    enc_mask = not bool(encm.all())
    out_mask = not bool(obm.all())

    cfg = {"any_bias": any_bias, "enc_mask": enc_mask, "out_mask": out_mask}
    key = (any_bias, enc_mask, out_mask)
    if key not in _CACHE:
        _CACHE[key] = _build(cfg)
    nc = _CACHE[key]

    # lstm weights, gate order [i, f, o, g]; g doubled so one exp computes all
    # gates: tanh(g) = 2*sigmoid(2g)-1
    lcat = np.concatenate([wih.T, whh.T], axis=0)       # (2*H(g), 4*H)
    perm = np.concatenate(
        [lcat[:, 0:H], lcat[:, H:2 * H], lcat[:, 3 * H:4 * H],
         2.0 * lcat[:, 2 * H:3 * H]],
        axis=1,
    )

    # shared (replicated) tensors
    import ml_dtypes
    bft = ml_dtypes.bfloat16
    shared = {
        "cembt": _pmajor(np.ascontiguousarray(cem.T), 2).astype(bft),
        "wqt": _pmajor(np.ascontiguousarray(aqw.T), 2).astype(bft),
        "wket": _pmajor(np.ascontiguousarray(akw[:, :D].T), 2).astype(bft),
        "wkh2": _pmajor(np.ascontiguousarray(akw[:, D:]), 2).astype(bft),
        "cwt": _pmajor(np.ascontiguousarray(cw.T), 4).astype(bft),
        "lwt": _pmajor(np.ascontiguousarray(perm), 4).astype(bft),
        "owt": _pmajor(np.ascontiguousarray(ow.T), 2).astype(bft),
        "onesq": np.ones((128, 128), bft),
    }
    if any_bias:
        bl = bih + bhh
        blp = np.concatenate(
            [bl[0:H], bl[H:2 * H], bl[3 * H:4 * H], 2.0 * bl[2 * H:3 * H]]
        )
        shared["bqc"] = _pmajor(aqb, 2)
        shared["bkc"] = _pmajor(akb, 2)
        shared["brow"] = np.concatenate(
            [cb, blp, obias]
        )[None, :].astype(bft)
        shared["ones"] = np.ones((1, BS), bft)
    if out_mask:
        shared["onest"] = np.ones((1, TT), bft)

    in_maps = []
    for c in range(NCORES):
        sl = slice(c * BS, (c + 1) * BS)
        e = enc[sl]                                    # (BS, L, D)
        tg_ = tgt[sl]                                  # (BS, TT, D)
        obs = ob[sl]                                   # (BS, NB, D)
        m = dict(shared)
        m["enc_r"] = np.ascontiguousarray(
            e.reshape(BS, 4, 128, D).transpose(2, 0, 1, 3)
        ).astype(bft)
        m["enct"] = np.ascontiguousarray(
            e.transpose(2, 0, 1).reshape(2, 128, BS, L).transpose(1, 0, 2, 3)
        ).astype(bft)
        m["tgtt"] = np.ascontiguousarray(
            tg_.transpose(2, 1, 0).reshape(2, 128, TT, BS).transpose(1, 0, 2, 3)
        ).astype(bft)
        m["obt"] = np.ascontiguousarray(
            obs.transpose(2, 0, 1).reshape(2, 128, BS, NB).transpose(1, 0, 2, 3)
        ).astype(ml_dtypes.bfloat16)
        if enc_mask:
            em = np.where(encm[sl], 0.0, -1e30).astype(np.float32)  # (BS, L)
            m["emadd"] = np.ascontiguousarray(
                em.reshape(BS, 4, 128).transpose(2, 0, 1)
            )
        if out_mask:
            m["bmr"] = np.where(obm[sl], 0.0, -1e30).astype(bft)
        in_maps.append(m)

    res = run_bass_kernel_spmd(nc, in_maps, list(range(NCORES)))
    outs = [res.results[c]["out"].reshape(BS, TT, V) for c in range(NCORES)]
    return np.concatenate(outs, axis=0).astype(np.float32)


# revision 33
# speedup vs baseline: 1.0103x; 1.0014x over previous
"""AttnRNN decoder kernel for trn2 (8 NeuronCores, data-parallel over batch).

Structure:
  host   : embedding gather, weight transposes, batch sharding (B=32 -> 4/core)
  device : phase0  seq_qT / m2T / keyT_enc precompute (t-independent terms)
           phase1  127 sequential steps, fully column-form (features on
                   partitions, batch=4 moving dim; no transposes; sigmoid via
                   exp+reciprocal so only one act-table set is used)
           phase2  hoisted vocab projection (bf16) + log_softmax + DMA out
"""

import numpy as np

import concourse.bass as bass
import concourse.bacc as bacc
import concourse.mybir as mybir
import concourse.tile as tile
from concourse.bass_utils import run_bass_kernel_spmd

F32 = mybir.dt.float32
BF16 = mybir.dt.bfloat16
AF = mybir.ActivationFunctionType

B, L, D, T = 32, 512, 256, 128
H, NCOM, NB = 256, 8000, 128
V = NCOM + NB
NCORES = 8
BS = B // NCORES          # 4 examples per core
TT = T - 1                # 127 decode steps

# vocab chunking for phase 2: 15x512 + 1x320 common, then 128 batched
VCH = [(i * 512, 512) for i in range(15)] + [(7680, 320)]


def _pmajor(x, nchunk):
    """(nchunk*128, ...) -> (128, nchunk, ...) partition-major."""
    s = x.shape
    return np.ascontiguousarray(
        x.reshape(nchunk, 128, *s[1:]).transpose(1, 0, *range(2, 1 + len(s)))
    )


def _build(cfg):
    """Build the single-core program (SPMD-replicated across 8 cores)."""
    nc = bacc.Bacc("TRN2", target_bir_lowering=False, debug=False)

    dr = {}

    def din(name, shape, dt=F32):
        dr[name] = nc.dram_tensor(name, list(shape), dt, kind="ExternalInput").ap()
        return dr[name]

    enc_r = din("enc_r", (128, BS, 4, D), BF16)    # enc[b, 128*lc+p, d]
    enct = din("enct", (128, 2, BS, L), BF16)      # enc[b, l, 128*dc+p]
    tgtt = din("tgtt", (128, 2, TT, BS), BF16)     # tgt[b, t, 128*dc+p]
    cembt = din("cembt", (128, 2, NCOM), BF16)     # common[v, 128*dc+p]
    obt = din("obt", (128, 2, BS, NB), BF16)       # batched[b, v, 128*dc+p]
    wqt = din("wqt", (128, 2, D), BF16)                  # Wq[e, 128*dc+p]
    wket = din("wket", (128, 2, D), BF16)
    wkh2 = din("wkh2", (128, 2, D), BF16)   # akw[e, 256+f] with e on partitions
    cwt = din("cwt", (128, 4, H), BF16)                  # combine_w[g, 128*fc+p]
    lwt = din("lwt", (128, 4, 4 * H), BF16)              # [W_ih.T ; W_hh.T], ifog->ifog perm
    owt = din("owt", (128, 2, D), BF16)
    onesq = din("onesq", (128, 128), BF16)
    if cfg["any_bias"]:
        bqc = din("bqc", (128, 2))
        bkc = din("bkc", (128, 2))
        brow = din("brow", (1, 2 * H + 4 * H + D), BF16)   # [bc(256), bl(1024, ifog-perm), bo(256)]
        ones = din("ones", (1, BS), BF16)
    if cfg["enc_mask"]:
        emadd = din("emadd", (128, BS, 4))
    if cfg["out_mask"]:
        bmr = din("bmr", (BS, NB), BF16)
        onest = din("onest", (1, TT), BF16)

    out = nc.dram_tensor("out", [BS, TT, V], F32, kind="ExternalOutput").ap()
    if cfg.get("dbg_lin"):
        dbglin = nc.dram_tensor("dbglin", [128, 2, BS, TT], BF16, kind="ExternalOutput").ap()
        dr_dbg = {}
        for nm_, sh_, dt_ in [("dbg_wt", [128, BS, 4, TT], BF16),
                              ("dbg_c", [128, 2, BS, TT], F32),
                              ("dbg_h", [128, 2, BS, TT], BF16),
                              ("dbg_scp", [128, BS, 4, TT], F32),
                              ("dbg_ket", [128, 2, TT, BS], BF16),
                              ("dbg_seqq", [128, 2, BS, L], BF16),
                              ("dbg_m2", [128, 2, BS, L], BF16)]:
            dr_dbg[nm_] = nc.dram_tensor(nm_, sh_, dt_, kind="ExternalOutput").ap()

    with tile.TileContext(nc) as tc:
        with (
            tc.tile_pool(name="const", bufs=1) as kc,
            tc.tile_pool(name="state", bufs=3) as stp,
        ):
            # ---- persistent SBUF loads (ordered to unblock phase 0/1) ----
            enct_sb0 = kc.tile([128, 2, BS, L], BF16)
            nc.sync.dma_start(enct_sb0[:], enct[:])
            wqt_sb = kc.tile([128, 2, D], BF16)
            nc.sync.dma_start(wqt_sb[:], wqt[:])
            wket_sb = kc.tile([128, 2, D], BF16)
            nc.sync.dma_start(wket_sb[:], wket[:])
            tgtt_sb = kc.tile([128, 2, TT, BS], BF16)
            nc.sync.dma_start(tgtt_sb[:], tgtt[:])
            wkh2_sb = kc.tile([128, 2, D], BF16)
            nc.sync.dma_start(wkh2_sb[:], wkh2[:])
            cwt_sb = kc.tile([128, 4, H], BF16)
            nc.sync.dma_start(cwt_sb[:], cwt[:])
            lwt_sb = kc.tile([128, 4, 4 * H], BF16)
            nc.sync.dma_start(lwt_sb[:], lwt[:])
            owt_sb = kc.tile([128, 2, D], BF16)
            nc.sync.dma_start(owt_sb[:], owt[:])
            onesq_sb = kc.tile([128, 128], BF16)
            nc.sync.dma_start(onesq_sb[:], onesq[:])
            enc_sb = kc.tile([128, BS, 4, D], BF16)
            nc.sync.dma_start(enc_sb[:], enc_r[:])
            # dummy exp: pulls the act-table load off the step-0 critical path
            warm = kc.tile([1, 1], F32)
            nc.scalar.activation(warm[:], onesq_sb[0:1, 0:1], AF.Exp)
            if cfg["any_bias"]:
                bqc_sb = kc.tile([128, 2], F32)
                nc.sync.dma_start(bqc_sb[:], bqc[:])
                bkc_sb = kc.tile([128, 2], F32)
                nc.sync.dma_start(bkc_sb[:], bkc[:])
                brow_sb = kc.tile([1, 2 * H + 4 * H + D], BF16)
                nc.sync.dma_start(brow_sb[:], brow[:])
                ones_sb = kc.tile([1, BS], BF16)
                nc.sync.dma_start(ones_sb[:], ones[:])
            if cfg["enc_mask"]:
                emadd_sb = kc.tile([128, BS, 4], F32)
                nc.sync.dma_start(emadd_sb[:], emadd[:])
            if cfg["out_mask"]:
                bmr_sb = kc.tile([BS, NB], BF16)
                nc.sync.dma_start(bmr_sb[:], bmr[:])
                onest_sb = kc.tile([1, TT], BF16)
                nc.sync.dma_start(onest_sb[:], onest[:])

            seqqt_sb = kc.tile([128, 2, BS, L], BF16)
            m2t_sb = kc.tile([128, 2, BS, L], BF16)
            ket_sb = kc.tile([128, 2, TT, BS], BF16)
            linT_sb = kc.tile([128, 2, BS, TT], BF16)

            # ---- phase 0: seq_qT, m2T, keyT_enc ----
            with (
                tc.tile_pool(name="enctp", bufs=1) as ep,
                tc.tile_pool(name="p0ps", bufs=2, space="PSUM") as p0,
            ):
                enct_sb = enct_sb0
                for b in range(BS):
                    for c in range(2):
                        ps = p0.tile([128, 512], F32)
                        for k in range(2):
                            nc.tensor.matmul(
                                ps[:],
                                wqt_sb[:, k, c * 128:(c + 1) * 128],
                                enct_sb[:, k, b, :],
                                start=(k == 0), stop=(k == 1),
                            )
                        if cfg["any_bias"]:
                            nc.scalar.activation(
                                seqqt_sb[:, c, b, :], ps[:], AF.Identity,
                                bias=bqc_sb[:, c:c + 1],
                            )
                        else:
                            nc.vector.tensor_copy(seqqt_sb[:, c, b, :], ps[:])
                for c in range(2):
                    ps = p0.tile([128, 512], F32)
                    for k in range(2):
                        nc.tensor.matmul(
                            ps[:, 0:TT * BS],
                            wket_sb[:, k, c * 128:(c + 1) * 128],
                            tgtt_sb[:, k, :, :],
                            start=(k == 0), stop=(k == 1),
                        )
                    if cfg["any_bias"]:
                        nc.scalar.activation(
                            ket_sb[:, c, :, :], ps[:, 0:TT * BS], AF.Identity,
                            bias=bkc_sb[:, c:c + 1],
                        )
                    else:
                        nc.vector.tensor_copy(ket_sb[:, c, :, :], ps[:, 0:TT * BS])

            # phase-2-only tensors: DMA'd after the phase-0 loads so they
            # don't delay phase 0/1 startup
            cembt_sb = kc.tile([128, 2, NCOM], BF16)
            nc.sync.dma_start(cembt_sb[:], cembt[:])
            obt_sb = kc.tile([128, 2, BS, NB], BF16)
            nc.sync.dma_start(obt_sb[:], obt[:])

            # ---- phase 1: 127 sequential steps, column form ----
            nb_ = 1 if cfg["any_bias"] else 0
            with (
                tc.tile_pool(name="scps", bufs=2, space="PSUM") as scps,
                tc.tile_pool(name="gpsp", bufs=1, space="PSUM") as gpsp,
                tc.tile_pool(name="smps", bufs=1, space="PSUM") as smps,
                tc.tile_pool(name="misc", bufs=1, space="PSUM") as mps,
                tc.tile_pool(name="m2ps", bufs=1, space="PSUM") as pm,
                tc.tile_pool(name="work", bufs=3) as sbw,
            ):
                hT_cur = None

                def lin_mms(hT):
                    lps = mps.tile([128, 2, BS], F32, tag="lps")
                    for c in range(2):
                        for k in range(2):
                            nc.tensor.matmul(
                                lps[:, c, :],
                                owt_sb[:, k, c * 128:(c + 1) * 128],
                                hT[:, k, :],
                                start=(k == 0), stop=(k == 1 and nb_ == 0),
                            )
                        if nb_:
                            nc.tensor.matmul(
                                lps[:, c, :],
                                brow_sb[0:1, 5 * H + c * 128:5 * H + (c + 1) * 128],
                                ones_sb[0:1, :],
                                start=False, stop=True,
                            )
                    return lps

                def m2_mms(b):
                    # m2T[:, :, b, :] is first needed by step b+1's h-half;
                    # computing example b's slice at the end of step b keeps
                    # step 0 from waiting on it and spreads the copies out
                    for c in range(2):
                        ps = pm.tile([128, 512], F32, tag="m2p")
                        for k in range(2):
                            nc.tensor.matmul(
                                ps[:],
                                wkh2_sb[:, k, c * 128:(c + 1) * 128],
                                seqqt_sb[:, k, b, :],
                                start=(k == 0), stop=(k == 1),
                            )
                        eng = nc.vector if c == 0 else nc.scalar
                        if c == 0:
                            nc.vector.tensor_copy(m2t_sb[:, c, b, :], ps[:])
                        else:
                            nc.scalar.activation(
                                m2t_sb[:, c, b, :], ps[:], AF.Identity
                            )

                for t in range(0 if cfg.get("skip_p1") else TT):
                    # -- scores (psum col [b, lc]): enc half has no h dep --
                    # NOTE: one accumulation group for the whole tile per step
                    # (start only on the first mm, stop only on the last):
                    # interleaved per-column groups in one psum zero region
                    # corrupt each other (start re-marks the region pending-
                    # zero, so open columns lose their partial sums).
                    scp = scps.tile([128, BS, 4], F32, tag="scp")
                    for b in range(BS):
                        for lc in range(4):
                            col = scp[:, b, lc:lc + 1]
                            for k in range(2):
                                nc.tensor.matmul(
                                    col,
                                    seqqt_sb[:, k, b, lc * 128:(lc + 1) * 128],
                                    ket_sb[:, k, t, b:b + 1],
                                    start=(b == 0 and lc == 0 and k == 0),
                                    stop=(t == 0 and b == BS - 1 and lc == 3 and k == 1),
                                )
                    if t > 0:
                        for b in range(BS):
                            for lc in range(4):
                                col = scp[:, b, lc:lc + 1]
                                for k in range(2):
                                    nc.tensor.matmul(
                                        col,
                                        m2t_sb[:, k, b, lc * 128:(lc + 1) * 128],
                                        hT_cur[:, k, b:b + 1],
                                        start=False,
                                        stop=(b == BS - 1 and lc == 3 and k == 1),
                                    )
                    # gates h-half early (off critical path); single
                    # accumulation group per step (see scores note)
                    gp = gpsp.tile([128, 8, BS], F32, tag="gp")
                    if t > 0:
                        for g in range(8):
                            for k in range(2):
                                nc.tensor.matmul(
                                    gp[:, g, :],
                                    lwt_sb[:, 2 + k, g * 128:(g + 1) * 128],
                                    hT_cur[:, k, :],
                                    start=(g == 0 and k == 0), stop=False,
                                )
                        # output projection for step t-1 (h_{t-1} ready now)
                        lps = lin_mms(hT_cur)
                        nc.vector.tensor_copy(linT_sb[:, :, :, t - 1], lps[:])

                    if cfg["enc_mask"]:
                        nc.vector.tensor_add(scp[:], scp[:], emadd_sb[:])

                    # -- softmax: exp -> sums (4 accumulated ones-matmuls,
                    #    replicated over partitions) -> recip -> scale --
                    wt = sbw.tile([128, BS, 4], BF16, tag="wt")
                    nc.scalar.activation(wt[:], scp[:], AF.Exp)
                    smp = smps.tile([128, 1, BS], F32, tag="smp")
                    for lc in range(4):
                        nc.tensor.matmul(
                            smp[:, 0, :], onesq_sb[:], wt[:, :, lc],
                            start=(lc == 0), stop=(lc == 3),
                        )
                    # attention context, column form
                    atp = mps.tile([128, 2, BS], F32, tag="atp")
                    for b in range(BS):
                        for dc in range(2):
                            for lc in range(4):
                                nc.tensor.matmul(
                                    atp[:, dc, b:b + 1],
                                    enc_sb[:, b, lc, dc * 128:(dc + 1) * 128],
                                    wt[:, b, lc:lc + 1],
                                    start=(lc == 0), stop=(lc == 3),
                                )
                    rb = sbw.tile([128, 1, BS], F32, tag="rb")
                    nc.vector.reciprocal(rb[:, 0, :], smp[:, 0, :])
                    ats = sbw.tile([128, 2, BS], BF16, tag="ats")
                    nc.vector.tensor_mul(
                        ats[:], atp[:], rb[:].broadcast_to([128, 2, BS])
                    )

                    # -- combine + relu --
                    cbp = mps.tile([128, 2, BS], F32, tag="cbp")
                    cl = [tgtt_sb[:, 0, t, :], tgtt_sb[:, 1, t, :],
                          ats[:, 0, :], ats[:, 1, :]]
                    for fc in range(2):
                        for k in range(4):
                            nc.tensor.matmul(
                                cbp[:, fc, :],
                                cwt_sb[:, k, fc * 128:(fc + 1) * 128],
                                cl[k],
                                start=(k == 0), stop=(k == 3 and nb_ == 0),
                            )
                        if nb_:
                            nc.tensor.matmul(
                                cbp[:, fc, :],
                                brow_sb[0:1, fc * 128:(fc + 1) * 128],
                                ones_sb[0:1, :],
                                start=False, stop=True,
                            )
                    cbT = sbw.tile([128, 2, BS], BF16, tag="cbT")
                    nc.vector.tensor_scalar_max(cbT[:], cbp[:], 0.0)

                    # -- LSTM gates, comb half (accumulates into gp) --
                    for g in range(8):
                        for k in range(2):
                            nc.tensor.matmul(
                                gp[:, g, :],
                                lwt_sb[:, k, g * 128:(g + 1) * 128],
                                cbT[:, k, :],
                                start=(t == 0 and g == 0 and k == 0),
                                stop=(g == 7 and k == 1 and nb_ == 0),
                            )
                        if nb_:
                            nc.tensor.matmul(
                                gp[:, g, :],
                                brow_sb[0:1, 2 * H + g * 128:2 * H + (g + 1) * 128],
                                ones_sb[0:1, :],
                                start=False, stop=(g == 7),
                            )

                    # -- gate nonlinearities, one exp for all gates:
                    #    sigmoid(x) = 1/(1+exp(-x)); tanh(g) = 2*sigmoid(2g)-1
                    #    (g-gate weights are pre-doubled on the host) --
                    sie = sbw.tile([128, 8, BS], F32, tag="sie")
                    nc.scalar.activation(sie[:], gp[:], AF.Exp, scale=-1.0)
                    si = sbw.tile([128, 8, BS], F32, tag="si")
                    nc.vector.tensor_scalar_add(si[:], sie[:], 1.0)
                    nc.vector.reciprocal(si[:], si[:])
                    # gc holds [tanh(g) | c_{t-1}] so si[i,f] multiplies both
                    # in a single op: m12 = [si_i*tg | si_f*c]
                    gc = gc_next if t > 0 else stp.tile([128, 4, BS], F32, tag="gc")
                    nc.vector.tensor_scalar(
                        gc[:, 0:2, :], si[:, 6:8, :], 2.0, -1.0,
                        mybir.AluOpType.mult, mybir.AluOpType.add,
                    )

                    # -- c/h update --
                    gc_next = stp.tile([128, 4, BS], F32, tag="gc")
                    if t > 0:
                        m12 = sbw.tile([128, 4, BS], F32, tag="m12")
                        nc.vector.tensor_mul(m12[:], si[:, 0:4, :], gc[:])
                        c_new = gc_next[:, 2:4, :]
                        nc.vector.tensor_add(c_new, m12[:, 0:2, :], m12[:, 2:4, :])
                    else:
                        c_new = gc_next[:, 2:4, :]
                        nc.vector.tensor_mul(c_new, si[:, 0:2, :], gc[:, 0:2, :])
                    tc_ = sbw.tile([128, 2, BS], F32, tag="tc")
                    nc.scalar.activation(tc_[:], c_new, AF.Tanh)
                    hT_new = stp.tile([128, 2, BS], BF16, tag="hstate")
                    nc.vector.tensor_mul(hT_new[:], si[:, 4:6, :], tc_[:])

                    if cfg.get("dbg_lin"):
                        nc.sync.dma_start(dr_dbg["dbg_h"][:, :, :, t], hT_new[:])
                        nc.sync.dma_start(dr_dbg["dbg_c"][:, :, :, t], c_new[:])
                        nc.sync.dma_start(dr_dbg["dbg_wt"][:, :, :, t], wt[:])
                        scpc = sbw.tile([128, BS, 4], F32, tag="scpdbg")
                        nc.vector.tensor_copy(scpc[:], scp[:])
                        nc.sync.dma_start(dr_dbg["dbg_scp"][:, :, :, t], scpc[:])

                    hT_cur = hT_new
                    if t == 0:
                        for b_ in range(BS):
                            m2_mms(b_)

                if not cfg.get("skip_p1"):
                    lps = lin_mms(hT_cur)
                    nc.vector.tensor_copy(linT_sb[:, :, :, TT - 1], lps[:])

            if cfg.get("dbg_lin"):
                nc.sync.dma_start(dbglin[:], linT_sb[:])
                nc.sync.dma_start(dr_dbg["dbg_ket"][:], ket_sb[:])
                nc.sync.dma_start(dr_dbg["dbg_seqq"][:], seqqt_sb[:])
                nc.sync.dma_start(dr_dbg["dbg_m2"][:], m2t_sb[:])
            # ---- phase 2: vocab projection + log_softmax ----
            with (
                tc.tile_pool(name="p2ps", bufs=2, space="PSUM") as p2,
                tc.tile_pool(name="p2ps_b", bufs=2, space="PSUM") as p2b,
                tc.tile_pool(name="ep2", bufs=1) as ep2,
                tc.tile_pool(name="outst", bufs=3) as osp,
                tc.tile_pool(name="sm2", bufs=2) as sm2,
            ):
                # groups of vocab units paired into (TT, 1024) psum tiles
                # spanning two banks, to amortize Act/DVE per-op overheads:
                # 7x [512|512] common, 1x [512|320] common, 1x [128] batched
                P2G = []
                for i in range(7):
                    P2G.append((i * 1024, [(0, 512, 1024 * i), (512, 512, 1024 * i + 512)]))
                P2G.append((7168, [(0, 512, 7168), (512, 320, 7680)]))
                P2G.append((NCOM, [(0, NB, None)]))
                ngr = len(P2G)

                def group_mms(b, g, ps):
                    voff, units = P2G[g]
                    wtot = 0
                    for (boff, w, coff) in units:
                        if coff is not None:
                            rhs = [cembt_sb[:, k, coff:coff + w] for k in range(2)]
                            masked = False
                        else:
                            rhs = [obt_sb[:, k, b, :] for k in range(2)]
                            masked = cfg["out_mask"]
                        for k in range(2):
                            nc.tensor.matmul(
                                ps[:, boff:boff + w],
                                linT_sb[:, k, b, :],
                                rhs[k],
                                start=(k == 0),
                                stop=(k == 1 and not masked),
                            )
                        if masked:
                            nc.tensor.matmul(
                                ps[:, boff:boff + w], onest_sb[0:1, :],
                                bmr_sb[b:b + 1, :],
                                start=False, stop=True,
                            )
                        wtot = boff + w
                    return wtot

                # pass 1: exp-sums only (Act); pass 2 recomputes the cheap
                # matmul and writes out = logits - log(sumexp) on DVE, so the
                # expensive Ln pass over the full vocab disappears.
                for b in range(0 if cfg.get("skip_p2") else BS):
                    ss = sm2.tile([TT, ngr], F32, tag="ss")
                    for g in range(ngr):
                        ps = p2.tile([TT, 1024], F32, tag="p2")
                        w = group_mms(b, g, ps)
                        # exp written in place over the psum logits (same
                        # dtype): only the accum_out row-sums are consumed
                        nc.scalar.activation(
                            ps[:, 0:w], ps[:, 0:w], AF.Exp,
                            accum_out=ss[:, g:g + 1],
                        )
                    st = sm2.tile([TT, 1], F32, tag="st")
                    nc.vector.reduce_sum(
                        st[:], ss[:], axis=mybir.AxisListType.X
                    )
                    lz = sm2.tile([TT, 1], F32, tag="lz")
                    nc.scalar.activation(lz[:], st[:], AF.Ln)
                    nlz = sm2.tile([TT, 1], F32, tag="nlz")
                    nc.vector.tensor_scalar_mul(nlz[:], lz[:], -1.0)
                    for g in range(ngr):
                        voff = P2G[g][0]
                        ps = p2b.tile([TT, 1024], F32, tag="p2b")
                        w = group_mms(b, g, ps)
                        ot = osp.tile([TT, 1024], F32, tag="ot", bufs=4)
                        if b == BS - 1 and g % 2 == 1:
                            # last example has no next-example exp stream to
                            # overlap with: split subs between DVE and Act
                            nc.scalar.activation(
                                ot[:, 0:w], ps[:, 0:w], AF.Identity, bias=nlz[:],
                            )
                        else:
                            nc.vector.tensor_scalar_sub(ot[:, 0:w], ps[:, 0:w], lz[:])
                        nc.sync.dma_start(
                            out[b, :, voff:voff + w], ot[:, 0:w]
                        )

    nc.compile()
    return nc


_CACHE = {}


def kernel(**inputs):
    inp = {k: np.asarray(v) for k, v in inputs.items()}
    enc = inp["encoder_outputs"].astype(np.float32)
    encm = inp["encoder_outputs_mask"]
    ob = inp["output_batched_encodings"].astype(np.float32)
    obm = inp["output_batched_encodings_mask"]
    idx = inp["target_idxs"]
    cem = inp["common_embedding"].astype(np.float32)
    akw = inp["attn_key_w"].astype(np.float32)
    akb = inp["attn_key_b"].astype(np.float32)
    aqw = inp["attn_query_w"].astype(np.float32)
    aqb = inp["attn_query_b"].astype(np.float32)
    cw = inp["combine_w"].astype(np.float32)
    cb = inp["combine_b"].astype(np.float32)
    wih = inp["lstm_w_ih"].astype(np.float32)
    whh = inp["lstm_w_hh"].astype(np.float32)
    bih = inp["lstm_b_ih"].astype(np.float32)
    bhh = inp["lstm_b_hh"].astype(np.float32)
    ow = inp["out_w"].astype(np.float32)
    obias = inp["out_b"].astype(np.float32)

    # teacher-forced embedding gather (host: data-dependent indexing)
    is_c = idx < NCOM
    cidx = np.clip(idx, 0, NCOM - 1)
    bidx = np.clip(idx - NCOM, 0, NB - 1)
    ge_c = cem[cidx]                                   # (B, T, D)
    ge_b = np.take_along_axis(ob, bidx[..., None], axis=1)
    tgt = np.where(is_c[..., None], ge_c, ge_b)[:, :TT, :].astype(np.float32)

    any_bias = bool(
        np.any(akb) or np.any(aqb) or np.any(cb) or np.any(bih)
        or np.any(bhh) or np.any(obias)
    )
    enc_mask = not bool(encm.all())
    out_mask = not bool(obm.all())

    cfg = {"any_bias": any_bias, "enc_mask": enc_mask, "out_mask": out_mask}
    key = (any_bias, enc_mask, out_mask)
    if key not in _CACHE:
        _CACHE[key] = _build(cfg)
    nc = _CACHE[key]

    # lstm weights, gate order [i, f, o, g]; g doubled so one exp computes all
    # gates: tanh(g) = 2*sigmoid(2g)-1
    lcat = np.concatenate([wih.T, whh.T], axis=0)       # (2*H(g), 4*H)
    perm = np.concatenate(
        [lcat[:, 0:H], lcat[:, H:2 * H], lcat[:, 3 * H:4 * H],
         2.0 * lcat[:, 2 * H:3 * H]],
        axis=1,
    )

    # shared (replicated) tensors
    import ml_dtypes
    bft = ml_dtypes.bfloat16
    shared = {
        "cembt": _pmajor(np.ascontiguousarray(cem.T), 2).astype(bft),
        "wqt": _pmajor(np.ascontiguousarray(aqw.T), 2).astype(bft),
        "wket": _pmajor(np.ascontiguousarray(akw[:, :D].T), 2).astype(bft),
        "wkh2": _pmajor(np.ascontiguousarray(akw[:, D:]), 2).astype(bft),
        "cwt": _pmajor(np.ascontiguousarray(cw.T), 4).astype(bft),
        "lwt": _pmajor(np.ascontiguousarray(perm), 4).astype(bft),
        "owt": _pmajor(np.ascontiguousarray(ow.T), 2).astype(bft),
        "onesq": np.ones((128, 128), bft),
    }
    if any_bias:
        bl = bih + bhh
        blp = np.concatenate(
            [bl[0:H], bl[H:2 * H], bl[3 * H:4 * H], 2.0 * bl[2 * H:3 * H]]
        )
        shared["bqc"] = _pmajor(aqb, 2)
        shared["bkc"] = _pmajor(akb, 2)
        shared["brow"] = np.concatenate(
            [cb, blp, obias]
        )[None, :].astype(bft)
        shared["ones"] = np.ones((1, BS), bft)
    if out_mask:
        shared["onest"] = np.ones((1, TT), bft)

    in_maps = []
    for c in range(NCORES):
        sl = slice(c * BS, (c + 1) * BS)
        e = enc[sl]                                    # (BS, L, D)
        tg_ = tgt[sl]                                  # (BS, TT, D)
        obs = ob[sl]                                   # (BS, NB, D)
        m = dict(shared)
        m["enc_r"] = np.ascontiguousarray(
            e.reshape(BS, 4, 128, D).transpose(2, 0, 1, 3)
        ).astype(bft)
        m["enct"] = np.ascontiguousarray(
            e.transpose(2, 0, 1).reshape(2, 128, BS, L).transpose(1, 0, 2, 3)
        ).astype(bft)
        m["tgtt"] = np.ascontiguousarray(
            tg_.transpose(2, 1, 0).reshape(2, 128, TT, BS).transpose(1, 0, 2, 3)
        ).astype(bft)
        m["obt"] = np.ascontiguousarray(
            obs.transpose(2, 0, 1).reshape(2, 128, BS, NB).transpose(1, 0, 2, 3)
        ).astype(ml_dtypes.bfloat16)
        if enc_mask:
            em = np.where(encm[sl], 0.0, -1e30).astype(np.float32)  # (BS, L)
            m["emadd"] = np.ascontiguousarray(
                em.reshape(BS, 4, 128).transpose(2, 0, 1)
            )
        if out_mask:
            m["bmr"] = np.where(obm[sl], 0.0, -1e30).astype(bft)
        in_maps.append(m)

    res = run_bass_kernel_spmd(nc, in_maps, list(range(NCORES)))
    outs = [res.results[c]["out"].reshape(BS, TT, V) for c in range(NCORES)]
    return np.concatenate(outs, axis=0).astype(np.float32)
